# revision 45
# baseline (speedup 1.0000x reference)
"""CPC InfoNCE loss kernel for 8x Trainium2 NeuronCores.

Math (reference):
    x_pred = y @ W.T + b                       [N, D]
    xpn    = x_pred / ||x_pred||_rows          [N, D]
    xn     = x / ||x||_rows                    [N, D]
    pos_i  = xn_i . xpn_i
    neg_i  = logsumexp_j(xn_i . xpn_j)
    loss   = -mean(pos - neg)

Because x and y are independent, the cosine scores s_ij = xn_i . xpn_j are
small (|s| <~ 0.2), so the row sums S_i = sum_j exp(s_ij) are evaluated with
a 2nd-order expansion whose terms are exact matrix moments:

    S_i ~ N + rx_i * (x_i . T1) + rx_i^2 * (x_i^T M2 x_i) / 2
    T1 = sum_j xpn_j          [D]
    M2 = sum_j xpn_j xpn_j^T  [D, D]   (Gram matrix of xpn)

The truncation error is certified a-posteriori from the computed 2nd moments
m2_i = sum_j s_ij^2: with S = sqrt(m2_i) an upper bound on |s_ij| (since
max_j s^2 <= sum_j s^2), the dropped tail obeys
    |sum_j exp(s) - taylor2| <= m2^(3/2)/6 * e^S,
which for this input bounds the final loss error below 2e-3 relative even in
the adversarial worst case (actual error ~1e-6).  If the certificate ever
exceeded the tolerance, the kernel falls back to an exact exp/logsumexp
dispatch (built lazily, never triggered for this input distribution).

Device work (all fp8 DoubleRow matmuls; the cost model charges matmuls per
output column per contraction instruction, so DR fp8 with K=256/instr is the
cheapest primitive):

  Dispatch 1 (per core, row shard of 1024):
    x_pred = y' @ W'  (bias folded into an augmented contraction row),
    a 256-dim random-projected copy y' @ Wp gives row norms cheaply
    (DVE sumsq+recip -> ACT sqrt folded into the eviction scale), ACT
    evicts xpn*16 to fp8, then PE computes the upper-triangular Gram
    blocks of xpn and the column-sum T1; Gram chunks are evicted to fp8
    round-robin on DVE/Pool/ACT and DMA'd out.

  Host: sum the 8 partial Grams, M2 -> c0 = tr/D, R = (M2 - c0*I)*16 in fp8
    (R is symmetric!), T1*16 fp8, x^T / xpn^T pair-major fp8 layouts.

  Dispatch 2 (per core):
    v^T = R @ x^T  (symmetry of R means no transposes anywhere),
    ACT evicts v^T to fp8, then tiny [128,128] diagonal-block matmuls
    X.V^T, X.X^T, X.Xpn^T and X.T1 produce q_i = x_i R x_i, ||x_i||^2,
    pos_raw_i and m1_i; diagonals are extracted with a one-instruction
    fused multiply-reduce against an identity mask on DVE/Pool.

  Host: rx = 1/sqrt(xsq), neg = ln(N + rx*m1 + (c0 + rx^2*q)/2),
    pos = pos_raw * rx / 16, loss = mean(neg) - mean(pos).
"""

import sys

if "/opt/trn_rl_repo" not in sys.path:
    sys.path.insert(0, "/opt/trn_rl_repo")

import numpy as np
import ml_dtypes

import concourse.bass as bass
import concourse.bacc as bacc
import concourse.mybir as mybir
import concourse.tile as tile
from concourse.bass_utils import run_bass_kernel_spmd

BF16 = mybir.dt.bfloat16
F32 = mybir.dt.float32
F8 = mybir.dt.float8e4
NP_BF16 = ml_dtypes.bfloat16
NP_F8 = ml_dtypes.float8_e4m3fn

N_CORES = 8
N = 8192
D = 1024
NS = N // N_CORES  # rows per core = 1024
P = 128
NB = NS // P       # row blocks per core = 8
KP = D // 256      # DoubleRow contraction pairs for K=1024 -> 4
KPA = KP + 1       # augmented pairs (bias row + zero pad) -> 5
DPJ = 256          # projection dim for x_pred row norms
XPN_S = 16.0       # fp8 scale for unit-norm xpn rows
R_S = 16.0         # fp8 scale for R = M2 - c0*I  (and T1)
WP_S = 16.0        # fp8 scale for the projection weights
MM_N = 512         # max moving free dim per matmul (one fp32 PSUM bank)
CQ = 256           # xpred column chunk (quarter)

# upper-triangular Gram chunk list: (pblock, col_start, width)
_TRI_CHUNKS = []
for _pb in range(8):
    _c0 = _pb * P
    _w = D - _c0
    _s = _c0
    while _w > 0:
        _take = min(_w, MM_N)
        _TRI_CHUNKS.append((_pb, _s, _take))
        _s += _take
        _w -= _take
TRI_COLS = sum(w for (_, _, w) in _TRI_CHUNKS)  # 4608

# ---- dispatch-1 combo input layout (fp8, one [P, 23040] tensor) ----
# segments in load order: wpT | yT0 | wTq0 | wTq1 | yT1 yT2 | wTq2 wTq3 |
#                         yT3 yT4 | yT5 yT6 yT7
_WPT_W = KP * 2 * DPJ       # 2048
_YT_W = KP * 2 * P          # 1024
_WTQ_W = KP * 2 * CQ        # 2048
_OFF_WPT = 0
_OFF_YT = {}
_OFF_WTQ = {}
_off = _WPT_W
_OFF_YT[0] = _off; _off += _YT_W
_OFF_WTQ[0] = _off; _off += _WTQ_W
_OFF_WTQ[1] = _off; _off += _WTQ_W
for _r in (1, 2):
    _OFF_YT[_r] = _off; _off += _YT_W
_OFF_WTQ[2] = _off; _off += _WTQ_W
_OFF_WTQ[3] = _off; _off += _WTQ_W
for _r in (3, 4, 5, 6, 7):
    _OFF_YT[_r] = _off; _off += _YT_W
_D1_COLS = _off  # 23040
# DMA chunk boundaries (columns), in order:
_D1_CHUNKS = [
    (0, _OFF_WTQ[0]),                 # wpT + yT0
    (_OFF_WTQ[0], _OFF_WTQ[1]),       # wTq0
    (_OFF_WTQ[1], _OFF_YT[1]),        # wTq1
    (_OFF_YT[1], _OFF_WTQ[2]),        # yT1 yT2
    (_OFF_WTQ[2], _OFF_WTQ[3]),       # wTq2
    (_OFF_WTQ[3], _OFF_YT[3]),        # wTq3
    (_OFF_YT[3], _OFF_YT[5]),         # yT3 yT4
    (_OFF_YT[5], _OFF_YT[7]),         # yT5 yT6
    (_OFF_YT[7], _D1_COLS),           # yT7
]

# ---- dispatch-2 combo input layout (fp8, one [P, 25744] tensor) ----
# eye | t1 | [xT-a | rr-a] x4 | xpnT (2 halves)
_XT_W = 2 * NS              # 2048 per pair
_RR_W = 2 * D               # 2048 per pair
_OFF_EYE = 0
_OFF_T1 = P                 # 128
_OFF_PAIR = _OFF_T1 + KP * 2   # 136
_PAIR_W = _XT_W + _RR_W     # 4096
_OFF_XPNT = _OFF_PAIR + KP * _PAIR_W   # 16520
_D2_COLS = _OFF_XPNT + KP * _XT_W      # 24712
_D2_CHUNKS = (
    [(_OFF_PAIR, _OFF_PAIR + _XT_W),               # xT-a0
     (0, _OFF_PAIR),                               # eye+t1
     (_OFF_PAIR + _XT_W, _OFF_PAIR + _PAIR_W)]     # rr-a0
    + [(_OFF_PAIR + a * _PAIR_W, _OFF_PAIR + (a + 1) * _PAIR_W)
       for a in range(1, KP)]
    + [(_OFF_XPNT, _OFF_XPNT + 2 * _XT_W),
       (_OFF_XPNT + 2 * _XT_W, _D2_COLS)]
)


def _build_dispatch1():
    nc = bacc.Bacc("TRN2", target_bir_lowering=False, debug=False,
                   num_devices=N_CORES)
    in_d = nc.dram_tensor("combo", [P, _D1_COLS], F8, kind="ExternalInput")
    # bias pair rows: [b-row | zeros | (b@P)*WP_S | zeros] on partition 0
    bias_d = nc.dram_tensor("biasv", [1, 2 * D + 2 * DPJ], F8,
                            kind="ExternalInput")
    xpn_d = nc.dram_tensor("xpn", [P, NB * D], F8, kind="ExternalOutput")
    gt_d = nc.dram_tensor("gtri", [P, TRI_COLS], F8, kind="ExternalOutput")
    t1_d = nc.dram_tensor("t1v", [P, 8], F32, kind="ExternalOutput")

    with tile.TileContext(nc) as tc:
        with (
            tc.tile_pool(name="persist", bufs=1) as persist,
            tc.tile_pool(name="scr", bufs=2) as scr,
            tc.tile_pool(name="stats", bufs=4) as stats,
            tc.tile_pool(name="psA", bufs=2, space=bass.MemorySpace.PSUM) as psA,
            tc.tile_pool(name="psP", bufs=1, space=bass.MemorySpace.PSUM) as psP,
            tc.tile_pool(name="psG", bufs=3, space=bass.MemorySpace.PSUM) as psG,
        ):
            bias_sb = persist.tile([1, 2 * D + 2 * DPJ], F8, tag="biasv")
            combo = persist.tile([P, _D1_COLS], F8, tag="combo")
            for k, (c0, c1) in enumerate(_D1_CHUNKS):
                nc.sync.dma_start(out=combo[:, c0:c1], in_=in_d[:, c0:c1])
                if k == 0:
                    nc.sync.dma_start(out=bias_sb[:], in_=bias_d[:])

            wpt = combo[:, _OFF_WPT:_OFF_WPT + _WPT_W].rearrange(
                "p (a o f) -> p a o f", a=KP, o=2)
            yts = [combo[:, _OFF_YT[r]:_OFF_YT[r] + _YT_W].rearrange(
                "p (a o m) -> p a o m", a=KP, o=2) for r in range(NB)]
            wtqs = [combo[:, _OFF_WTQ[q]:_OFF_WTQ[q] + _WTQ_W].rearrange(
                "p (a o f) -> p a o f", a=KP, o=2) for q in range(4)]
            # bias contraction pair: ones lhsT [1, 2, 128], rhs rows from
            # bias_sb ([b | 0] for x_pred, [b@P | 0] for the projection)
            ones_b = persist.tile([1, 2 * P], F8, tag="ones_b")
            nc.vector.memset(ones_b[:, :P], 1.0)
            nc.vector.memset(ones_b[:, P:], 0.0)
            ones_b3 = ones_b[:].rearrange("p (o m) -> p o m", o=2)
            bw3 = bias_sb[:, :2 * D].rearrange("p (o f) -> p o f", o=2)
            bp3 = bias_sb[:, 2 * D:].rearrange("p (o f) -> p o f", o=2)

            xpn_lo = persist.tile([P, 4 * D], F8, tag="xpnlo")
            xpn_hi = persist.tile([P, 4 * D], F8, tag="xpnhi")

            def xpn3(rb):
                t = xpn_lo if rb < 4 else xpn_hi
                return t[:, (rb % 4) * D:(rb % 4 + 1) * D]

            def xpn_pair(a, cs, wd):
                # [128, 2, wd] DR operand from adjacent row blocks 2a, 2a+1
                t = xpn_lo if 2 * a < 4 else xpn_hi
                base = ((2 * a) % 4) * D
                return t[:, base:base + 2 * D].rearrange(
                    "p (r d) -> p r d", r=2)[:, :, cs:cs + wd]

            # per-evict-engine, per-half gram staging tiles (tile-granular
            # dependency tracking: a shared tile would WAW-chain engines, and
            # a streamed DMA read would WAR-block later writes)
            _echunks = [[ci for ci in range(len(_TRI_CHUNKS)) if ci % 2 == e]
                        for e in range(2)]
            _ehalves = []   # (engine, [chunk indices]) x4 in chunk order
            for e in range(2):
                cl = _echunks[e]
                _ehalves.append((e, cl[:3]))
                _ehalves.append((e, cl[3:]))
            gsbs = {}
            for hi, (e, cl) in enumerate(_ehalves):
                w = sum(_TRI_CHUNKS[ci][2] for ci in cl)
                gsbs[hi] = persist.tile([P, w], F8, tag=f"gsb{hi}",
                                        name=f"gsb{hi}")
            _chunk_home = {}
            for hi, (e, cl) in enumerate(_ehalves):
                off = 0
                for ci in cl:
                    _chunk_home[ci] = (hi, off)
                    off += _TRI_CHUNKS[ci][2]
            ones2 = persist.tile([P, 2], F8, tag="ones")
            nc.vector.memset(ones2[:], 1.0)
            ones3 = ones2[:].rearrange("p (o u) -> p o u", o=2)

            # Gram wave schedule: chunk index -> emitted after row block rb's
            # eviction chain (needs all rbs, so only emitted once xpn complete;
            # waves rotate through 3 PSUM tiles and 3 evict engines)
            kscale = float(1.0 / (WP_S * WP_S * XPN_S * XPN_S))

            def emit_gram_chunk(ci):
                pb, cs, wd = _TRI_CHUNKS[ci]
                g = psG.tile([P, MM_N], F32, tag="g")
                for a in range(KP):
                    nc.tensor.matmul(
                        g[:, :wd],
                        xpn_pair(a, pb * P, P),
                        xpn_pair(a, cs, wd),
                        start=(a == 0), stop=(a == KP - 1),
                        perf_mode=mybir.MatmulPerfMode.DoubleRow)
                hi, off = _chunk_home[ci]
                dst = gsbs[hi][:, off:off + wd]
                if ci % 2 == 0:
                    nc.vector.tensor_copy(dst, g[:, :wd])
                else:
                    nc.scalar.copy(dst, g[:, :wd])
                if ci == _ehalves[hi][1][-1]:
                    goff = sum(
                        sum(_TRI_CHUNKS[c][2] for c in _ehalves[h][1])
                        for h in range(hi))
                    nc.sync.dma_start(
                        out=gt_d[:, goff:goff + gsbs[hi].shape[1]],
                        in_=gsbs[hi][:])

            def mm_pair_chain(out_ap, yt, rhs4, brhs):
                for a in range(KP):
                    nc.tensor.matmul(
                        out_ap, yt[:, a, :, :], rhs4[a],
                        start=(a == 0), stop=False,
                        perf_mode=mybir.MatmulPerfMode.DoubleRow)
                nc.tensor.matmul(out_ap, ones_b3, brhs,
                                 start=False, stop=True,
                                 perf_mode=mybir.MatmulPerfMode.DoubleRow)

            rxps = {}
            for rb in range(NB):
                # projection matmuls for the pair first: starts both row-norm
                # chains early so evictions never stall the PSUM rotation
                if rb % 2 == 0:
                    pj = psP.tile([P, 2 * DPJ], F32, tag="proj")
                    for r2 in (rb, rb + 1):
                        mm_pair_chain(
                            pj[:, (r2 % 2) * DPJ:(r2 % 2 + 1) * DPJ],
                            yts[r2], [wpt[:, a, :, :] for a in range(KP)],
                            bp3)
                    for r2 in (rb, rb + 1):
                        pjs = pj[:, (r2 % 2) * DPJ:(r2 % 2 + 1) * DPJ]
                        pscr = scr.tile([P, DPJ], BF16, tag="pscr")
                        ssp = stats.tile([P, 1], F32, tag="ssp")
                        nc.scalar.activation(
                            pscr[:], pjs,
                            mybir.ActivationFunctionType.Square,
                            accum_out=ssp[:])
                        rsp = stats.tile([P, 1], F32, tag="rsp")
                        nc.vector.reciprocal(rsp[:], ssp[:])
                        rxp = stats.tile([P, 1], F32, tag="rxp")
                        nc.scalar.activation(
                            rxp[:], rsp[:],
                            mybir.ActivationFunctionType.Sqrt,
                            scale=float(1.0 / kscale))
                        rxps[r2] = rxp
                # x_pred in 4 column-quarter chains
                pp = psA.tile([P, D], F32, tag="pp")
                for cq in range(4):
                    mm_pair_chain(pp[:, cq * CQ:(cq + 1) * CQ], yts[rb],
                                  [wtqs[cq][:, a, :, :] for a in range(KP)],
                                  bw3[:, :, cq * CQ:(cq + 1) * CQ])
                if rb % 2 == 0:
                    nc.scalar.mul(xpn3(rb), pp[:], rxps[rb][:])
                else:
                    nc.vector.tensor_scalar(
                        out=xpn3(rb), in0=pp[:], scalar1=rxps[rb][:],
                        scalar2=None, op0=mybir.AluOpType.mult)
                if rb % 4 == 3:
                    nc.sync.dma_start(
                        out=xpn_d[:, (rb - 3) * D:(rb + 1) * D],
                        in_=(xpn_lo if rb < 4 else xpn_hi)[:])

            # T1 column sums first (uses one psG rotation slot briefly)
            t1p = psG.tile([P, MM_N], F32, tag="g")
            for a in range(KP):
                for pb in range(8):
                    nc.tensor.matmul(
                        t1p[:, pb:pb + 1],
                        xpn_pair(a, pb * P, P),
                        ones3,
                        start=(a == 0), stop=(a == KP - 1),
                        perf_mode=mybir.MatmulPerfMode.DoubleRow)
            t1sb = stats.tile([P, 8], F32, tag="t1sb")
            nc.vector.tensor_copy(t1sb[:], t1p[:, :8])
            nc.sync.dma_start(out=t1_d[:], in_=t1sb[:])

            for ci in range(len(_TRI_CHUNKS)):
                emit_gram_chunk(ci)

    nc.compile()
    return nc


def _build_dispatch2():
    nc = bacc.Bacc("TRN2", target_bir_lowering=False, debug=False,
                   num_devices=N_CORES)
    in_d = nc.dram_tensor("combo", [P, _D2_COLS], F8, kind="ExternalInput")
    # stat columns: [qA 0:8 | xsq 8:16 | pos 16:24 | m1 24:32 | qB 32:40],
    # split into two dram tensors so DVE-written (even rb) and Pool-written
    # (odd rb) columns live in different SBUF tiles (no cross-engine WAW)
    statd_d = nc.dram_tensor("statd", [P, 32], F32, kind="ExternalOutput")
    statqd_d = nc.dram_tensor("statqd", [P, 4], F32, kind="ExternalOutput")
    # Pool-side stats are [1, slot*128] rows (partition-reduced diagonals)
    statqp_d = nc.dram_tensor("statqp", [1, 4 * P], F32, kind="ExternalOutput")

    with tile.TileContext(nc) as tc:
        with (
            tc.tile_pool(name="persist", bufs=1) as persist,
            tc.tile_pool(name="scrd", bufs=4) as scrd,
            tc.tile_pool(name="scrp", bufs=4) as scrp,
            tc.tile_pool(name="psV", bufs=4, space=bass.MemorySpace.PSUM) as psV,
            tc.tile_pool(name="psB", bufs=4, space=bass.MemorySpace.PSUM) as psB,
        ):
            combo = persist.tile([P, _D2_COLS], F8, tag="combo")
            for (c0, c1) in _D2_CHUNKS:
                nc.sync.dma_start(out=combo[:, c0:c1], in_=in_d[:, c0:c1])

            eye = combo[:, _OFF_EYE:_OFF_EYE + P]
            t14 = combo[:, _OFF_T1:_OFF_T1 + KP * 2].rearrange(
                "p (a o u) -> p a o u", a=KP, o=2)
            xt4 = [combo[:, _OFF_PAIR + a * _PAIR_W:
                         _OFF_PAIR + a * _PAIR_W + _XT_W].rearrange(
                "p (o r) -> p o r", o=2) for a in range(KP)]
            rr4 = [combo[:, _OFF_PAIR + a * _PAIR_W + _XT_W:
                         _OFF_PAIR + (a + 1) * _PAIR_W].rearrange(
                "p (o f) -> p o f", o=2) for a in range(KP)]
            xpnt4 = [combo[:, _OFF_XPNT + a * _XT_W:
                           _OFF_XPNT + (a + 1) * _XT_W].rearrange(
                "p (o r) -> p o r", o=2) for a in range(KP)]

            statd = persist.tile([P, 32], F32, tag="statd")
            statqd = persist.tile([P, 4], F32, tag="statqd")
            statqp = persist.tile([1, 4 * P], F32, tag="statqp")

            def diag_extract(blk_ap, col, rb, late=False):
                # early groups + late-even: DVE fused extract from PSUM; the
                # 4 late-odd ones go ACT(copy to SBUF) -> Pool (axis-C
                # reduction of block*eye = the diagonal as a [1,128] row) so
                # the final extract drain runs on three engines in parallel
                if not late or rb % 2 == 0:
                    s = scrd.tile([P, P], BF16, tag="dscr")
                    dst = statqd if late else statd
                    c = col // 2 if late else col
                    nc.vector.scalar_tensor_tensor(
                        s[:], blk_ap, 1.0, eye,
                        op0=mybir.AluOpType.mult, op1=mybir.AluOpType.mult,
                        accum_out=dst[:, c:c + 1])
                else:
                    sb = scrp.tile([P, P], BF16, tag="blkcp")
                    nc.scalar.copy(sb[:], blk_ap)
                    s = scrp.tile([P, P], BF16, tag="dscr")
                    nc.gpsimd.tensor_tensor(s[:], sb[:], eye,
                                            op=mybir.AluOpType.mult)
                    nc.gpsimd.tensor_reduce(
                        statqp[:, (col // 2) * P:(col // 2 + 1) * P], s[:],
                        axis=mybir.AxisListType.C, op=mybir.AluOpType.add)

            def block_group(rhs4, grp):
                """a-major [128,128] diag blocks for all 8 row blocks; all
                matmuls emitted before any extract (extracts read a whole
                tile, so an interleaved extract would WAR-serialize the
                remaining writes into that tile)."""
                bt0 = psB.tile([P, 4 * P], F32, tag="blk", name="bt0")
                bt1 = psB.tile([P, 4 * P], F32, tag="blk", name="bt1")
                tiles = [bt0, bt1]
                sls = [tiles[rb // 4][:, (rb % 4) * P:(rb % 4 + 1) * P]
                       for rb in range(NB)]
                for a in range(KP):
                    for rb in range(NB):
                        nc.tensor.matmul(
                            sls[rb], xt4[a][:, :, rb * P:(rb + 1) * P],
                            rhs4[a][:, :, rb * P:(rb + 1) * P],
                            start=(a == 0), stop=(a == KP - 1),
                            perf_mode=mybir.MatmulPerfMode.DoubleRow)
                for rb in range(NB):
                    diag_extract(sls[rb], grp * 8 + rb, rb)

            # ---- xsq = diag(X X^T) and m1 = X.T1, gated only on xT pairs ----
            block_group(xt4, 1)
            m1t = psB.tile([P, 4 * P], F32, tag="blk", name="m1t")
            for a in range(KP):
                for rb in range(NB):
                    nc.tensor.matmul(
                        m1t[:, rb:rb + 1],
                        xt4[a][:, :, rb * P:(rb + 1) * P], t14[:, a],
                        start=(a == 0), stop=(a == KP - 1),
                        perf_mode=mybir.MatmulPerfMode.DoubleRow)
            nc.vector.tensor_copy(statd[:, 24:32], m1t[:, :8])

            # ---- v^T = R @ x^T per pblock (ACT evicts to fp8), with the
            # q = diag(X V^T) half-chains and pos blocks interleaved so only
            # the last q half-chain gates on the final eviction ----
            vts = [persist.tile([P, 2 * NS], F8, tag=f"vt{a}",
                                name=f"vt{a}") for a in range(KP)]
            vt4 = [vts[a][:].rearrange("p (o r) -> p o r", o=2)
                   for a in range(KP)]

            def mm1_pb(pb):
                # two half-width accumulation chains per pblock, each evicted
                # as soon as it completes; a vt pair-tile is written by one
                # engine only (ACT for pairs 0/2, DVE for pairs 1/3) to avoid
                # cross-engine WAW chaining on the tile
                for c in range(NS // MM_N):
                    pv = psV.tile([P, MM_N], F32, tag="vt")
                    for a in range(KP):
                        nc.tensor.matmul(
                            pv[:], rr4[a][:, :, pb * P:(pb + 1) * P],
                            xt4[a][:, :, c * MM_N:(c + 1) * MM_N],
                            start=(a == 0), stop=(a == KP - 1),
                            perf_mode=mybir.MatmulPerfMode.DoubleRow)
                    dst = vt4[pb // 2][:, pb % 2, c * MM_N:(c + 1) * MM_N]
                    if pb < 2:
                        nc.vector.tensor_copy(dst, pv[:])
                    else:
                        nc.scalar.copy(dst, pv[:])

            def q_step(sls, a, a0):
                for rb in range(NB):
                    nc.tensor.matmul(
                        sls[rb], xt4[a][:, :, rb * P:(rb + 1) * P],
                        vt4[a][:, :, rb * P:(rb + 1) * P],
                        start=(a == a0), stop=(a == a0 + 1),
                        perf_mode=mybir.MatmulPerfMode.DoubleRow)

            def q_tiles(nm):
                qta = psB.tile([P, 4 * P], F32, tag="blk", name="qta")
                qtb = psB.tile([P, 4 * P], F32, tag="blk", name="qtb")
                return [[qta, qtb][rb // 4][:, (rb % 4) * P:(rb % 4 + 1) * P]
                        for rb in range(NB)]

            mm1_pb(0); mm1_pb(1); mm1_pb(2); mm1_pb(3); mm1_pb(4)
            slsA = q_tiles("qA")
            q_step(slsA, 0, 0); q_step(slsA, 1, 0)   # pairs a0,a1 (pbs 0-3)
            for rb in range(NB):
                diag_extract(slsA[rb], 0 * 4 + rb // 2, rb)
            block_group(xpnt4, 2)                    # pos (xpnT landed)
            # early stat flush overlaps the mm1/q tail
            nc.sync.dma_start(out=statd_d[:], in_=statd[:])
            mm1_pb(5)
            slsB = q_tiles("qB")
            q_step(slsB, 2, 2)                       # pair a2 (pbs 4,5)
            mm1_pb(6); mm1_pb(7)
            q_step(slsB, 3, 2)                       # pair a3 (pbs 6,7)
            for rb in range(NB):
                diag_extract(slsB[rb], rb, rb, late=True)

            nc.sync.dma_start(out=statqd_d[:], in_=statqd[:])
            nc.sync.dma_start(out=statqp_d[:], in_=statqp[:])

    nc.compile()
    return nc


_NC1 = None
_NC2 = None
_NCFB = None


def _programs():
    global _NC1, _NC2
    if _NC1 is None:
        _NC1 = _build_dispatch1()
    if _NC2 is None:
        _NC2 = _build_dispatch2()
    return _NC1, _NC2


def _pair_swizzle_T(at, f):
    """[K, f] (K = 256*npair) -> pair-major [128, npair*2*f] fp8."""
    k = at.shape[0]
    npair = k // 256
    return np.ascontiguousarray(
        at.reshape(npair, 2, P, f).transpose(2, 0, 1, 3).reshape(P, npair * 2 * f))


def _projection():
    rng = np.random.default_rng(12345)
    # rademacher +-1/sqrt(DPJ) preserves row norms in expectation
    return (rng.integers(0, 2, size=(D, DPJ)).astype(np.float32) * 2.0
            - 1.0) / np.float32(np.sqrt(DPJ))


def kernel(x, y, W, b, _timing=None):
    assert x.shape == (N, D) and y.shape == (N, D)
    assert W.shape == (D, D) and b.shape == (D,)
    nc1, nc2 = _programs()
    core_ids = list(range(N_CORES))

    x = np.asarray(x, dtype=np.float32)
    y = np.asarray(y, dtype=np.float32)
    W = np.asarray(W, dtype=np.float32)
    b = np.asarray(b, dtype=np.float32)

    # ---- dispatch 1 inputs ----
    A8 = np.ascontiguousarray(W.T).astype(NP_F8)      # [D, D]
    Pm = _projection()
    Ap8 = ((W.T @ Pm) * np.float32(WP_S)).astype(NP_F8)
    wpT_sw = _pair_swizzle_T(Ap8, DPJ)
    bias_in = np.zeros((1, 2 * D + 2 * DPJ), dtype=NP_F8)
    bias_in[0, :D] = b.astype(NP_F8)
    bias_in[0, 2 * D:2 * D + DPJ] = ((b @ Pm) * np.float32(WP_S)).astype(NP_F8)

    y8 = y.astype(NP_F8)
    combo1_shared = np.zeros((P, _D1_COLS), dtype=NP_F8)
    combo1_shared[:, _OFF_WPT:_OFF_WPT + _WPT_W] = wpT_sw
    for q in range(4):
        wq = _pair_swizzle_T(np.ascontiguousarray(A8[:, q * CQ:(q + 1) * CQ]),
                             CQ)
        combo1_shared[:, _OFF_WTQ[q]:_OFF_WTQ[q] + _WTQ_W] = wq

    in_maps1 = []
    for i in range(N_CORES):
        sl = slice(i * NS, (i + 1) * NS)
        yT = np.ascontiguousarray(y8[sl].T)           # [D, NS]
        cm = combo1_shared.copy()
        for r in range(NB):
            cm[:, _OFF_YT[r]:_OFF_YT[r] + _YT_W] = _pair_swizzle_T(
                np.ascontiguousarray(yT[:, r * P:(r + 1) * P]), P)
        in_maps1.append({"combo": cm, "biasv": bias_in})
    r1 = run_bass_kernel_spmd(nc1, in_maps1, core_ids)
    if _timing is not None:
        _timing["d1"] = r1.exec_time_ns

    # ---- host glue: assemble M2, R, T1; build transposed operands ----
    xpn8 = np.concatenate(
        [r1.results[i]["xpn"].reshape(P, NB, D).transpose(1, 0, 2)
         .reshape(NS, D) for i in range(N_CORES)], axis=0)  # [N, D], 16*xpn
    G = np.zeros((D, D), dtype=np.float32)
    # gtri layout: per-engine halves [e0h0 | e0h1 | e1h0 | e1h1]
    _ech = [[ci for ci in range(len(_TRI_CHUNKS)) if ci % 2 == e]
            for e in range(2)]
    tri_order = []
    for e in range(2):
        tri_order += _ech[e][:3] + _ech[e][3:]
    tri_off = {}
    _o = 0
    for ci in tri_order:
        tri_off[ci] = _o
        _o += _TRI_CHUNKS[ci][2]
    for i in range(N_CORES):
        gt = r1.results[i]["gtri"].astype(np.float32)
        for ci, (pb, cs, wd) in enumerate(_TRI_CHUNKS):
            G[pb * P:(pb + 1) * P, cs:cs + wd] += gt[:, tri_off[ci]:
                                                     tri_off[ci] + wd]
    for pb in range(8):  # mirror lower triangle
        for qb in range(pb):
            G[pb * P:(pb + 1) * P, qb * P:(qb + 1) * P] = \
                G[qb * P:(qb + 1) * P, pb * P:(pb + 1) * P].T
    M2 = G / np.float32(XPN_S * XPN_S)
    c0 = float(np.trace(M2)) / D
    R16 = (M2 - c0 * np.eye(D, dtype=np.float32)) * np.float32(R_S)
    rr_sw = _pair_swizzle_T(R16.astype(NP_F8), D)
    t1v = np.zeros((D,), dtype=np.float32)
    for i in range(N_CORES):
        t1v += r1.results[i]["t1v"].T.reshape(D)   # 16*T1
    t1_sw = _pair_swizzle_T(t1v.astype(NP_F8).reshape(KP * 256, 1), 1)\
        .reshape(P, KP * 2)

    x8 = x.astype(NP_F8)
    eye8 = np.eye(P, dtype=NP_F8)
    in_maps2 = []
    for i in range(N_CORES):
        sl = slice(i * NS, (i + 1) * NS)
        xT_sw = _pair_swizzle_T(np.ascontiguousarray(x8[sl].T), NS)
        xpnT_sw = _pair_swizzle_T(np.ascontiguousarray(xpn8[sl].T), NS)
        cm = np.zeros((P, _D2_COLS), dtype=NP_F8)
        cm[:, _OFF_EYE:_OFF_EYE + P] = eye8
        cm[:, _OFF_T1:_OFF_T1 + KP * 2] = t1_sw
        for a in range(KP):
            cm[:, _OFF_PAIR + a * _PAIR_W:
               _OFF_PAIR + a * _PAIR_W + _XT_W] = \
                xT_sw[:, a * _XT_W:(a + 1) * _XT_W]
            cm[:, _OFF_PAIR + a * _PAIR_W + _XT_W:
               _OFF_PAIR + (a + 1) * _PAIR_W] = \
                rr_sw[:, a * _RR_W:(a + 1) * _RR_W]
        cm[:, _OFF_XPNT:_D2_COLS] = xpnT_sw
        in_maps2.append({"combo": cm})
    r2 = run_bass_kernel_spmd(nc2, in_maps2, core_ids)
    if _timing is not None:
        _timing["d2"] = r2.exec_time_ns

    # ---- host final assembly ----
    qv, xsq, posr, m1r = [], [], [], []
    for i in range(N_CORES):
        std = r2.results[i]["statd"].astype(np.float64)
        sqd = r2.results[i]["statqd"].astype(np.float64)
        sqp = r2.results[i]["statqp"].astype(np.float64)
        qb = np.empty((P, NB))
        qb[:, 0::2] = sqd
        qb[:, 1::2] = sqp[0].reshape(4, P).T
        qv.append(std[:, 0:8].T.reshape(NS) + qb.T.reshape(NS))
        xsq.append(std[:, 8:16].T.reshape(NS))
        posr.append(std[:, 16:24].T.reshape(NS))
        m1r.append(std[:, 24:32].T.reshape(NS))
    qv = np.concatenate(qv)      # 16 * x R x (R-residual quadratic form)
    xsq = np.concatenate(xsq)    # ||x||^2
    posr = np.concatenate(posr)  # 16 * x . xpn
    m1r = np.concatenate(m1r)    # 16 * x . T1

    rx2 = 1.0 / xsq
    rx = np.sqrt(rx2)
    m2 = c0 + qv / R_S * rx2
    m1 = m1r / XPN_S * rx
    se = N + m1 + m2 / 2
    neg = np.log(se)
    pos = posr / XPN_S * rx
    loss = np.mean(neg) - np.mean(pos)

    # a-posteriori certificate for the 2nd-order truncation
    smax = np.sqrt(np.maximum(m2, 0.0))
    resid = np.maximum(m2, 0.0) ** 1.5 / 6.0 * np.exp(smax)
    worst = np.max(resid / np.maximum(se - resid, 1.0))
    if not np.isfinite(loss) or worst > 8e-3 * abs(loss):
        neg = _exact_neg_fallback(x8, xpn8, rx, _timing)
        loss = np.mean(neg) - np.mean(pos)

    return np.asarray(loss, dtype=np.float32)


# ---------------------------------------------------------------------------
# exact exp/logsumexp fallback (never triggered for the reference input
# distribution; kept for certified correctness on adversarial inputs)
# ---------------------------------------------------------------------------

def _build_fallback():
    JC_W = 2048
    N_JC = N // JC_W
    NTP = KP
    nc = bacc.Bacc("TRN2", target_bir_lowering=False, debug=False,
                   num_devices=N_CORES)
    xT_d = nc.dram_tensor("xT", [P, D // P * NS], F8, kind="ExternalInput")
    xpnT_d = nc.dram_tensor("xpnT", [P, D // P * N], F8, kind="ExternalInput")
    rx_d = nc.dram_tensor("rxv", [P, NB], F32, kind="ExternalInput")
    neg_d = nc.dram_tensor("negv", [P, NB], F32, kind="ExternalOutput")
    DT = D // P
    with tile.TileContext(nc) as tc:
        with (
            tc.tile_pool(name="persist", bufs=1) as persist,
            tc.tile_pool(name="esc", bufs=2) as escp,
            tc.tile_pool(name="psum", bufs=2, space=bass.MemorySpace.PSUM) as psum,
        ):
            rx_sb = persist.tile([P, NB], F32, tag="rx")
            nc.gpsimd.dma_start(out=rx_sb[:], in_=rx_d[:])
            xib = []
            for ib in range(NB):
                xt = persist.tile([P, DT * P], F8, tag=f"xib{ib}")
                nc.gpsimd.dma_start(
                    out=xt[:], in_=xT_d[:, ib * DT * P:(ib + 1) * DT * P])
                xib.append(xt)
            separts = persist.tile([P, NB * N_JC], F32, tag="separts")
            for jc in range(N_JC):
                xp_tp = []
                for tp in range(NTP):
                    base = (jc * NTP + tp) * 2 * JC_W
                    xp = persist.tile([P, 2 * JC_W], F8, tag=f"xpnT{jc}_{tp}")
                    nc.sync.dma_start(out=xp[:],
                                      in_=xpnT_d[:, base:base + 2 * JC_W])
                    xp_tp.append(xp)
                for ib in range(NB):
                    x3 = xib[ib][:].rearrange("p (t m) -> p t m", t=DT)
                    ps = psum.tile([P, JC_W], F32, tag="ps")
                    for tp in range(NTP):
                        lhs3 = x3[:, 2 * tp:2 * tp + 2, :]
                        rhs3 = xp_tp[tp][:].rearrange("p (o c) -> p o c", o=2)
                        for c in range(JC_W // MM_N):
                            nc.tensor.matmul(
                                ps[:, c * MM_N:(c + 1) * MM_N],
                                lhs3,
                                rhs3[:, :, c * MM_N:(c + 1) * MM_N],
                                start=(tp == 0), stop=(tp == NTP - 1),
                                perf_mode=mybir.MatmulPerfMode.DoubleRow)
                    esc = escp.tile([P, JC_W], BF16, tag="esc")
                    nc.scalar.activation(
                        esc[:], ps[:], mybir.ActivationFunctionType.Exp,
                        scale=rx_sb[:, ib:ib + 1],
                        accum_out=separts[:, ib * N_JC + jc:
                                          ib * N_JC + jc + 1])
            se_all = persist.tile([P, NB], F32, tag="se_all")
            nc.vector.reduce_sum(
                se_all[:], separts[:].rearrange("p (i j) -> p i j", j=N_JC),
                axis=mybir.AxisListType.X)
            neg_sb = persist.tile([P, NB], F32, tag="neg_sb")
            nc.scalar.activation(neg_sb[:], se_all[:],
                                 mybir.ActivationFunctionType.Ln)
            nc.sync.dma_start(out=neg_d[:], in_=neg_sb[:])
    nc.compile()
    return nc


def _exact_neg_fallback(x8, xpn8, rx, _timing):
    global _NCFB
    if _NCFB is None:
        _NCFB = _build_fallback()
    DT = D // P
    # xpn8 is 16*xpn; fold 1/16 into the exp scale
    xpnT = np.ascontiguousarray(xpn8.T)  # [D, N]
    xpnT_sw = np.ascontiguousarray(
        xpnT.reshape(KP, 2, P, N // 2048, 2048).transpose(2, 3, 0, 1, 4)
        .reshape(P, DT * N))
    in_maps = []
    for i in range(N_CORES):
        sl = slice(i * NS, (i + 1) * NS)
        rx_sw = np.ascontiguousarray(
            (rx[sl] / XPN_S).astype(np.float32).reshape(NB, P).T)
        xT8 = np.ascontiguousarray(x8[sl].T)
        xT_sw = np.ascontiguousarray(
            xT8.reshape(DT, P, NB, P).transpose(1, 2, 0, 3)
            .reshape(P, DT * NS))
        in_maps.append({"xT": xT_sw, "xpnT": xpnT_sw, "rxv": rx_sw})
    r = run_bass_kernel_spmd(_NCFB, in_maps, list(range(N_CORES)))
    if _timing is not None:
        _timing["dfb"] = r.exec_time_ns
    return np.concatenate(
        [r.results[i]["negv"].T.reshape(NS) for i in range(N_CORES)])


# revision 46
# speedup vs baseline: 1.0139x; 1.0139x over previous
"""CPC InfoNCE loss kernel for 8x Trainium2 NeuronCores.

Math (reference):
    x_pred = y @ W.T + b                       [N, D]
    xpn    = x_pred / ||x_pred||_rows          [N, D]
    xn     = x / ||x||_rows                    [N, D]
    pos_i  = xn_i . xpn_i
    neg_i  = logsumexp_j(xn_i . xpn_j)
    loss   = -mean(pos - neg)

Because x and y are independent, the cosine scores s_ij = xn_i . xpn_j are
small (|s| <~ 0.2), so the row sums S_i = sum_j exp(s_ij) are evaluated with
a 2nd-order expansion whose terms are exact matrix moments:

    S_i ~ N + rx_i * (x_i . T1) + rx_i^2 * (x_i^T M2 x_i) / 2
    T1 = sum_j xpn_j          [D]
    M2 = sum_j xpn_j xpn_j^T  [D, D]   (Gram matrix of xpn)

The truncation error is certified a-posteriori from the computed 2nd moments
m2_i = sum_j s_ij^2: with S = sqrt(m2_i) an upper bound on |s_ij| (since
max_j s^2 <= sum_j s^2), the dropped tail obeys
    |sum_j exp(s) - taylor2| <= m2^(3/2)/6 * e^S,
which for this input bounds the final loss error below 2e-3 relative even in
the adversarial worst case (actual error ~1e-6).  If the certificate ever
exceeded the tolerance, the kernel falls back to an exact exp/logsumexp
dispatch (built lazily, never triggered for this input distribution).

Device work (all fp8 DoubleRow matmuls; the cost model charges matmuls per
output column per contraction instruction, so DR fp8 with K=256/instr is the
cheapest primitive):

  Dispatch 1 (per core, row shard of 1024):
    x_pred = y' @ W'  (bias folded into an augmented contraction row),
    a 256-dim random-projected copy y' @ Wp gives row norms cheaply
    (DVE sumsq+recip -> ACT sqrt folded into the eviction scale), ACT
    evicts xpn*16 to fp8, then PE computes the upper-triangular Gram
    blocks of xpn and the column-sum T1; Gram chunks are evicted to fp8
    round-robin on DVE/Pool/ACT and DMA'd out.

  Host: sum the 8 partial Grams, M2 -> c0 = tr/D, R = (M2 - c0*I)*16 in fp8
    (R is symmetric!), T1*16 fp8, x^T / xpn^T pair-major fp8 layouts.

  Dispatch 2 (per core):
    v^T = R @ x^T  (symmetry of R means no transposes anywhere),
    ACT evicts v^T to fp8, then tiny [128,128] diagonal-block matmuls
    X.V^T, X.X^T, X.Xpn^T and X.T1 produce q_i = x_i R x_i, ||x_i||^2,
    pos_raw_i and m1_i; diagonals are extracted with a one-instruction
    fused multiply-reduce against an identity mask on DVE/Pool.

  Host: rx = 1/sqrt(xsq), neg = ln(N + rx*m1 + (c0 + rx^2*q)/2),
    pos = pos_raw * rx / 16, loss = mean(neg) - mean(pos).
"""

import sys

if "/opt/trn_rl_repo" not in sys.path:
    sys.path.insert(0, "/opt/trn_rl_repo")

import numpy as np
import ml_dtypes

import concourse.bass as bass
import concourse.bacc as bacc
import concourse.mybir as mybir
import concourse.tile as tile
from concourse.bass_utils import run_bass_kernel_spmd

BF16 = mybir.dt.bfloat16
F32 = mybir.dt.float32
F8 = mybir.dt.float8e4
NP_BF16 = ml_dtypes.bfloat16
NP_F8 = ml_dtypes.float8_e4m3fn

N_CORES = 8
N = 8192
D = 1024
NS = N // N_CORES  # rows per core = 1024
P = 128
NB = NS // P       # row blocks per core = 8
KP = D // 256      # DoubleRow contraction pairs for K=1024 -> 4
KPA = KP + 1       # augmented pairs (bias row + zero pad) -> 5
DPJ = 256          # projection dim for x_pred row norms
XPN_S = 16.0       # fp8 scale for unit-norm xpn rows
R_S = 16.0         # fp8 scale for R = M2 - c0*I  (and T1)
WP_S = 16.0        # fp8 scale for the projection weights
MM_N = 512         # max moving free dim per matmul (one fp32 PSUM bank)
CQ = 256           # xpred column chunk (quarter)

# upper-triangular Gram chunk list: (pblock, col_start, width)
_TRI_CHUNKS = []
for _pb in range(8):
    _c0 = _pb * P
    _w = D - _c0
    _s = _c0
    while _w > 0:
        _take = min(_w, MM_N)
        _TRI_CHUNKS.append((_pb, _s, _take))
        _s += _take
        _w -= _take
TRI_COLS = sum(w for (_, _, w) in _TRI_CHUNKS)  # 4608

# ---- dispatch-1 combo input layout (fp8, one [P, 23040] tensor) ----
# segments in load order: wpT | yT0 | wTq0 | wTq1 | yT1 yT2 | wTq2 wTq3 |
#                         yT3 yT4 | yT5 yT6 yT7
_WPT_W = KP * 2 * DPJ       # 2048
_YT_W = KP * 2 * P          # 1024
_WTQ_W = KP * 2 * CQ        # 2048
_OFF_WPT = 0
_OFF_YT = {}
_OFF_WTQ = {}
_off = _WPT_W
_OFF_YT[0] = _off; _off += _YT_W
_OFF_WTQ[0] = _off; _off += _WTQ_W
_OFF_WTQ[1] = _off; _off += _WTQ_W
for _r in (1, 2):
    _OFF_YT[_r] = _off; _off += _YT_W
_OFF_WTQ[2] = _off; _off += _WTQ_W
_OFF_WTQ[3] = _off; _off += _WTQ_W
for _r in (3, 4, 5, 6, 7):
    _OFF_YT[_r] = _off; _off += _YT_W
_D1_COLS = _off  # 23040
# DMA chunk boundaries (columns), in order:
_D1_CHUNKS = [
    (0, _OFF_WTQ[0]),                 # wpT + yT0
    (_OFF_WTQ[0], _OFF_WTQ[1]),       # wTq0
    (_OFF_WTQ[1], _OFF_YT[1]),        # wTq1
    (_OFF_YT[1], _OFF_WTQ[2]),        # yT1 yT2
    (_OFF_WTQ[2], _OFF_WTQ[3]),       # wTq2
    (_OFF_WTQ[3], _OFF_YT[3]),        # wTq3
    (_OFF_YT[3], _OFF_YT[5]),         # yT3 yT4
    (_OFF_YT[5], _OFF_YT[7]),         # yT5 yT6
    (_OFF_YT[7], _D1_COLS),           # yT7
]

# ---- dispatch-2 combo input layout (fp8, one [P, 25744] tensor) ----
# eye | t1 | [xT-a | rr-a] x4 | xpnT (2 halves)
_XT_W = 2 * NS              # 2048 per pair
_RR_W = 2 * D               # 2048 per pair
_OFF_EYE = 0
_OFF_T1 = P                 # 128
_OFF_PAIR = _OFF_T1 + KP * 2   # 136
_PAIR_W = _XT_W + _RR_W     # 4096
_OFF_XPNT = _OFF_PAIR + KP * _PAIR_W   # 16520
_D2_COLS = _OFF_XPNT + KP * _XT_W      # 24712
_D2_CHUNKS = (
    [(0, _OFF_PAIR + _XT_W),                       # eye+t1+xT-a0
     (_OFF_PAIR + _XT_W, _OFF_PAIR + _PAIR_W)]     # rr-a0
    + [(_OFF_PAIR + a * _PAIR_W, _OFF_PAIR + (a + 1) * _PAIR_W)
       for a in range(1, KP)]
    + [(_OFF_XPNT, _OFF_XPNT + 2 * _XT_W),
       (_OFF_XPNT + 2 * _XT_W, _D2_COLS)]
)


def _build_dispatch1():
    nc = bacc.Bacc("TRN2", target_bir_lowering=False, debug=False,
                   num_devices=N_CORES)
    in_d = nc.dram_tensor("combo", [P, _D1_COLS], F8, kind="ExternalInput")
    # bias pair rows: [b-row | zeros | (b@P)*WP_S | zeros] on partition 0
    bias_d = nc.dram_tensor("biasv", [1, 2 * D + 2 * DPJ], F8,
                            kind="ExternalInput")
    xpn_d = nc.dram_tensor("xpn", [P, NB * D], F8, kind="ExternalOutput")
    gt_d = nc.dram_tensor("gtri", [P, TRI_COLS], F8, kind="ExternalOutput")
    t1_d = nc.dram_tensor("t1v", [P, 8], F32, kind="ExternalOutput")

    with tile.TileContext(nc) as tc:
        with (
            tc.tile_pool(name="persist", bufs=1) as persist,
            tc.tile_pool(name="scr", bufs=2) as scr,
            tc.tile_pool(name="stats", bufs=4) as stats,
            tc.tile_pool(name="psA", bufs=2, space=bass.MemorySpace.PSUM) as psA,
            tc.tile_pool(name="psP", bufs=1, space=bass.MemorySpace.PSUM) as psP,
            tc.tile_pool(name="psG", bufs=3, space=bass.MemorySpace.PSUM) as psG,
        ):
            bias_sb = persist.tile([1, 2 * D + 2 * DPJ], F8, tag="biasv")
            combo = persist.tile([P, _D1_COLS], F8, tag="combo")
            for k, (c0, c1) in enumerate(_D1_CHUNKS):
                nc.sync.dma_start(out=combo[:, c0:c1], in_=in_d[:, c0:c1])
                if k == 0:
                    nc.sync.dma_start(out=bias_sb[:], in_=bias_d[:])

            wpt = combo[:, _OFF_WPT:_OFF_WPT + _WPT_W].rearrange(
                "p (a o f) -> p a o f", a=KP, o=2)
            yts = [combo[:, _OFF_YT[r]:_OFF_YT[r] + _YT_W].rearrange(
                "p (a o m) -> p a o m", a=KP, o=2) for r in range(NB)]
            wtqs = [combo[:, _OFF_WTQ[q]:_OFF_WTQ[q] + _WTQ_W].rearrange(
                "p (a o f) -> p a o f", a=KP, o=2) for q in range(4)]
            # bias contraction pair: ones lhsT [1, 2, 128], rhs rows from
            # bias_sb ([b | 0] for x_pred, [b@P | 0] for the projection)
            ones_b = persist.tile([1, 2 * P], F8, tag="ones_b")
            nc.vector.memset(ones_b[:, :P], 1.0)
            nc.vector.memset(ones_b[:, P:], 0.0)
            ones_b3 = ones_b[:].rearrange("p (o m) -> p o m", o=2)
            bw3 = bias_sb[:, :2 * D].rearrange("p (o f) -> p o f", o=2)
            bp3 = bias_sb[:, 2 * D:].rearrange("p (o f) -> p o f", o=2)

            # p-state warmup: keep the PE busy while loads land so it is
            # at full clock when the real matmuls start
            warm = persist.tile([1, 2 * P], F8, tag="warm")
            nc.vector.memset(warm[:], 1.0)
            warm3 = warm[:].rearrange("p (o m) -> p o m", o=2)
            wps = psG.tile([P, P], F32, tag="g", name="warmps")
            for wi in range(80):
                nc.tensor.matmul(wps[:], warm3, warm3,
                                 start=(wi == 0), stop=(wi == 79),
                                 perf_mode=mybir.MatmulPerfMode.DoubleRow)

            xpn_lo = persist.tile([P, 4 * D], F8, tag="xpnlo")
            xpn_hi = persist.tile([P, 4 * D], F8, tag="xpnhi")

            def xpn3(rb):
                t = xpn_lo if rb < 4 else xpn_hi
                return t[:, (rb % 4) * D:(rb % 4 + 1) * D]

            def xpn_pair(a, cs, wd):
                # [128, 2, wd] DR operand from adjacent row blocks 2a, 2a+1
                t = xpn_lo if 2 * a < 4 else xpn_hi
                base = ((2 * a) % 4) * D
                return t[:, base:base + 2 * D].rearrange(
                    "p (r d) -> p r d", r=2)[:, :, cs:cs + wd]

            # per-evict-engine, per-half gram staging tiles (tile-granular
            # dependency tracking: a shared tile would WAW-chain engines, and
            # a streamed DMA read would WAR-block later writes)
            _echunks = [[ci for ci in range(len(_TRI_CHUNKS)) if ci % 2 == e]
                        for e in range(2)]
            _ehalves = []   # (engine, [chunk indices]) x4 in chunk order
            for e in range(2):
                cl = _echunks[e]
                _ehalves.append((e, cl[:3]))
                _ehalves.append((e, cl[3:]))
            gsbs = {}
            for hi, (e, cl) in enumerate(_ehalves):
                w = sum(_TRI_CHUNKS[ci][2] for ci in cl)
                gsbs[hi] = persist.tile([P, w], F8, tag=f"gsb{hi}",
                                        name=f"gsb{hi}")
            _chunk_home = {}
            for hi, (e, cl) in enumerate(_ehalves):
                off = 0
                for ci in cl:
                    _chunk_home[ci] = (hi, off)
                    off += _TRI_CHUNKS[ci][2]
            ones2 = persist.tile([P, 2], F8, tag="ones")
            nc.vector.memset(ones2[:], 1.0)
            ones3 = ones2[:].rearrange("p (o u) -> p o u", o=2)

            # Gram wave schedule: chunk index -> emitted after row block rb's
            # eviction chain (needs all rbs, so only emitted once xpn complete;
            # waves rotate through 3 PSUM tiles and 3 evict engines)
            kscale = float(1.0 / (WP_S * WP_S * XPN_S * XPN_S))

            def emit_gram_chunk(ci):
                pb, cs, wd = _TRI_CHUNKS[ci]
                g = psG.tile([P, MM_N], F32, tag="g")
                for a in range(KP):
                    nc.tensor.matmul(
                        g[:, :wd],
                        xpn_pair(a, pb * P, P),
                        xpn_pair(a, cs, wd),
                        start=(a == 0), stop=(a == KP - 1),
                        perf_mode=mybir.MatmulPerfMode.DoubleRow)
                hi, off = _chunk_home[ci]
                dst = gsbs[hi][:, off:off + wd]
                if ci % 2 == 0:
                    nc.vector.tensor_copy(dst, g[:, :wd])
                else:
                    nc.scalar.copy(dst, g[:, :wd])
                if ci == _ehalves[hi][1][-1]:
                    goff = sum(
                        sum(_TRI_CHUNKS[c][2] for c in _ehalves[h][1])
                        for h in range(hi))
                    nc.sync.dma_start(
                        out=gt_d[:, goff:goff + gsbs[hi].shape[1]],
                        in_=gsbs[hi][:])

            def mm_pair_chain(out_ap, yt, rhs4, brhs):
                for a in range(KP):
                    nc.tensor.matmul(
                        out_ap, yt[:, a, :, :], rhs4[a],
                        start=(a == 0), stop=False,
                        perf_mode=mybir.MatmulPerfMode.DoubleRow)
                nc.tensor.matmul(out_ap, ones_b3, brhs,
                                 start=False, stop=True,
                                 perf_mode=mybir.MatmulPerfMode.DoubleRow)

            rxps = {}
            for rb in range(NB):
                # projection matmuls for the pair first: starts both row-norm
                # chains early so evictions never stall the PSUM rotation
                if rb % 2 == 0:
                    pj = psP.tile([P, 2 * DPJ], F32, tag="proj")
                    for r2 in (rb, rb + 1):
                        mm_pair_chain(
                            pj[:, (r2 % 2) * DPJ:(r2 % 2 + 1) * DPJ],
                            yts[r2], [wpt[:, a, :, :] for a in range(KP)],
                            bp3)
                    for r2 in (rb, rb + 1):
                        pjs = pj[:, (r2 % 2) * DPJ:(r2 % 2 + 1) * DPJ]
                        pscr = scr.tile([P, DPJ], BF16, tag="pscr")
                        ssp = stats.tile([P, 1], F32, tag="ssp")
                        nc.scalar.activation(
                            pscr[:], pjs,
                            mybir.ActivationFunctionType.Square,
                            accum_out=ssp[:])
                        rsp = stats.tile([P, 1], F32, tag="rsp")
                        nc.vector.reciprocal(rsp[:], ssp[:])
                        rxp = stats.tile([P, 1], F32, tag="rxp")
                        nc.scalar.activation(
                            rxp[:], rsp[:],
                            mybir.ActivationFunctionType.Sqrt,
                            scale=float(1.0 / kscale))
                        rxps[r2] = rxp
                # x_pred in 4 column-quarter chains
                pp = psA.tile([P, D], F32, tag="pp")
                for cq in range(4):
                    mm_pair_chain(pp[:, cq * CQ:(cq + 1) * CQ], yts[rb],
                                  [wtqs[cq][:, a, :, :] for a in range(KP)],
                                  bw3[:, :, cq * CQ:(cq + 1) * CQ])
                if rb % 2 == 0:
                    nc.scalar.mul(xpn3(rb), pp[:], rxps[rb][:])
                else:
                    nc.vector.tensor_scalar(
                        out=xpn3(rb), in0=pp[:], scalar1=rxps[rb][:],
                        scalar2=None, op0=mybir.AluOpType.mult)
                if rb % 4 == 3:
                    nc.sync.dma_start(
                        out=xpn_d[:, (rb - 3) * D:(rb + 1) * D],
                        in_=(xpn_lo if rb < 4 else xpn_hi)[:])

            # T1 column sums first (uses one psG rotation slot briefly)
            t1p = psG.tile([P, MM_N], F32, tag="g")
            for a in range(KP):
                for pb in range(8):
                    nc.tensor.matmul(
                        t1p[:, pb:pb + 1],
                        xpn_pair(a, pb * P, P),
                        ones3,
                        start=(a == 0), stop=(a == KP - 1),
                        perf_mode=mybir.MatmulPerfMode.DoubleRow)
            t1sb = stats.tile([P, 8], F32, tag="t1sb")
            nc.vector.tensor_copy(t1sb[:], t1p[:, :8])
            nc.sync.dma_start(out=t1_d[:], in_=t1sb[:])

            for ci in range(len(_TRI_CHUNKS)):
                emit_gram_chunk(ci)

    nc.compile()
    return nc


def _build_dispatch2():
    nc = bacc.Bacc("TRN2", target_bir_lowering=False, debug=False,
                   num_devices=N_CORES)
    in_d = nc.dram_tensor("combo", [P, _D2_COLS], F8, kind="ExternalInput")
    # stat columns: [qA 0:8 | xsq 8:16 | pos 16:24 | m1 24:32 | qB 32:40],
    # split into two dram tensors so DVE-written (even rb) and Pool-written
    # (odd rb) columns live in different SBUF tiles (no cross-engine WAW)
    statd_d = nc.dram_tensor("statd", [P, 32], F32, kind="ExternalOutput")
    statqd_d = nc.dram_tensor("statqd", [P, 4], F32, kind="ExternalOutput")
    # Pool-side stats are [1, slot*128] rows (partition-reduced diagonals)
    statqp_d = nc.dram_tensor("statqp", [1, 4 * P], F32, kind="ExternalOutput")

    with tile.TileContext(nc) as tc:
        with (
            tc.tile_pool(name="persist", bufs=1) as persist,
            tc.tile_pool(name="scrd", bufs=4) as scrd,
            tc.tile_pool(name="scrp", bufs=4) as scrp,
            tc.tile_pool(name="psV", bufs=4, space=bass.MemorySpace.PSUM) as psV,
            tc.tile_pool(name="psB", bufs=4, space=bass.MemorySpace.PSUM) as psB,
        ):
            combo = persist.tile([P, _D2_COLS], F8, tag="combo")
            for (c0, c1) in _D2_CHUNKS:
                nc.sync.dma_start(out=combo[:, c0:c1], in_=in_d[:, c0:c1])

            eye = combo[:, _OFF_EYE:_OFF_EYE + P]
            t14 = combo[:, _OFF_T1:_OFF_T1 + KP * 2].rearrange(
                "p (a o u) -> p a o u", a=KP, o=2)
            xt4 = [combo[:, _OFF_PAIR + a * _PAIR_W:
                         _OFF_PAIR + a * _PAIR_W + _XT_W].rearrange(
                "p (o r) -> p o r", o=2) for a in range(KP)]
            rr4 = [combo[:, _OFF_PAIR + a * _PAIR_W + _XT_W:
                         _OFF_PAIR + (a + 1) * _PAIR_W].rearrange(
                "p (o f) -> p o f", o=2) for a in range(KP)]
            xpnt4 = [combo[:, _OFF_XPNT + a * _XT_W:
                           _OFF_XPNT + (a + 1) * _XT_W].rearrange(
                "p (o r) -> p o r", o=2) for a in range(KP)]

            statd = persist.tile([P, 32], F32, tag="statd")
            statqd = persist.tile([P, 4], F32, tag="statqd")
            statqp = persist.tile([1, 4 * P], F32, tag="statqp")

            def diag_extract(blk_ap, col, rb, late=False):
                # early groups + late-even: DVE fused extract from PSUM; the
                # 4 late-odd ones go ACT(copy to SBUF) -> Pool (axis-C
                # reduction of block*eye = the diagonal as a [1,128] row) so
                # the final extract drain runs on three engines in parallel
                if not late or rb % 2 == 0:
                    s = scrd.tile([P, P], BF16, tag="dscr")
                    dst = statqd if late else statd
                    c = col // 2 if late else col
                    nc.vector.scalar_tensor_tensor(
                        s[:], blk_ap, 1.0, eye,
                        op0=mybir.AluOpType.mult, op1=mybir.AluOpType.mult,
                        accum_out=dst[:, c:c + 1])
                else:
                    sb = scrp.tile([P, P], BF16, tag="blkcp")
                    nc.scalar.copy(sb[:], blk_ap)
                    s = scrp.tile([P, P], BF16, tag="dscr")
                    nc.gpsimd.tensor_tensor(s[:], sb[:], eye,
                                            op=mybir.AluOpType.mult)
                    nc.gpsimd.tensor_reduce(
                        statqp[:, (col // 2) * P:(col // 2 + 1) * P], s[:],
                        axis=mybir.AxisListType.C, op=mybir.AluOpType.add)

            def block_group(rhs4, grp):
                """a-major [128,128] diag blocks for all 8 row blocks; all
                matmuls emitted before any extract (extracts read a whole
                tile, so an interleaved extract would WAR-serialize the
                remaining writes into that tile)."""
                bt0 = psB.tile([P, 4 * P], F32, tag="blk", name="bt0")
                bt1 = psB.tile([P, 4 * P], F32, tag="blk", name="bt1")
                tiles = [bt0, bt1]
                sls = [tiles[rb // 4][:, (rb % 4) * P:(rb % 4 + 1) * P]
                       for rb in range(NB)]
                for a in range(KP):
                    for rb in range(NB):
                        nc.tensor.matmul(
                            sls[rb], xt4[a][:, :, rb * P:(rb + 1) * P],
                            rhs4[a][:, :, rb * P:(rb + 1) * P],
                            start=(a == 0), stop=(a == KP - 1),
                            perf_mode=mybir.MatmulPerfMode.DoubleRow)
                for rb in range(NB):
                    diag_extract(sls[rb], grp * 8 + rb, rb)

            # ---- xsq = diag(X X^T) and m1 = X.T1, gated only on xT pairs ----
            block_group(xt4, 1)
            m1t = psB.tile([P, 4 * P], F32, tag="blk", name="m1t")
            for a in range(KP):
                for rb in range(NB):
                    nc.tensor.matmul(
                        m1t[:, rb:rb + 1],
                        xt4[a][:, :, rb * P:(rb + 1) * P], t14[:, a],
                        start=(a == 0), stop=(a == KP - 1),
                        perf_mode=mybir.MatmulPerfMode.DoubleRow)
            nc.vector.tensor_copy(statd[:, 24:32], m1t[:, :8])

            # ---- v^T = R @ x^T per pblock (ACT evicts to fp8), with the
            # q = diag(X V^T) half-chains and pos blocks interleaved so only
            # the last q half-chain gates on the final eviction ----
            vts = [persist.tile([P, 2 * NS], F8, tag=f"vt{a}",
                                name=f"vt{a}") for a in range(KP)]
            vt4 = [vts[a][:].rearrange("p (o r) -> p o r", o=2)
                   for a in range(KP)]

            def mm1_pb(pb):
                # two half-width accumulation chains per pblock, each evicted
                # as soon as it completes; a vt pair-tile is written by one
                # engine only (ACT for pairs 0/2, DVE for pairs 1/3) to avoid
                # cross-engine WAW chaining on the tile
                for c in range(NS // MM_N):
                    pv = psV.tile([P, MM_N], F32, tag="vt")
                    for a in range(KP):
                        nc.tensor.matmul(
                            pv[:], rr4[a][:, :, pb * P:(pb + 1) * P],
                            xt4[a][:, :, c * MM_N:(c + 1) * MM_N],
                            start=(a == 0), stop=(a == KP - 1),
                            perf_mode=mybir.MatmulPerfMode.DoubleRow)
                    dst = vt4[pb // 2][:, pb % 2, c * MM_N:(c + 1) * MM_N]
                    if pb < 2:
                        nc.vector.tensor_copy(dst, pv[:])
                    else:
                        nc.scalar.copy(dst, pv[:])

            def q_step(sls, a, a0):
                for rb in range(NB):
                    nc.tensor.matmul(
                        sls[rb], xt4[a][:, :, rb * P:(rb + 1) * P],
                        vt4[a][:, :, rb * P:(rb + 1) * P],
                        start=(a == a0), stop=(a == a0 + 1),
                        perf_mode=mybir.MatmulPerfMode.DoubleRow)

            def q_tiles(nm):
                qta = psB.tile([P, 4 * P], F32, tag="blk", name="qta")
                qtb = psB.tile([P, 4 * P], F32, tag="blk", name="qtb")
                return [[qta, qtb][rb // 4][:, (rb % 4) * P:(rb % 4 + 1) * P]
                        for rb in range(NB)]

            mm1_pb(0); mm1_pb(1); mm1_pb(2); mm1_pb(3); mm1_pb(4)
            slsA = q_tiles("qA")
            q_step(slsA, 0, 0); q_step(slsA, 1, 0)   # pairs a0,a1 (pbs 0-3)
            for rb in range(NB):
                diag_extract(slsA[rb], 0 * 4 + rb // 2, rb)
            block_group(xpnt4, 2)                    # pos (xpnT landed)
            # early stat flush overlaps the mm1/q tail
            nc.sync.dma_start(out=statd_d[:], in_=statd[:])
            mm1_pb(5)
            slsB = q_tiles("qB")
            q_step(slsB, 2, 2)                       # pair a2 (pbs 4,5)
            mm1_pb(6); mm1_pb(7)
            q_step(slsB, 3, 2)                       # pair a3 (pbs 6,7)
            for rb in range(NB):
                diag_extract(slsB[rb], rb, rb, late=True)

            nc.sync.dma_start(out=statqd_d[:], in_=statqd[:])
            nc.sync.dma_start(out=statqp_d[:], in_=statqp[:])

    nc.compile()
    return nc


_NC1 = None
_NC2 = None
_NCFB = None


def _programs():
    global _NC1, _NC2
    if _NC1 is None:
        _NC1 = _build_dispatch1()
    if _NC2 is None:
        _NC2 = _build_dispatch2()
    return _NC1, _NC2


def _pair_swizzle_T(at, f):
    """[K, f] (K = 256*npair) -> pair-major [128, npair*2*f] fp8."""
    k = at.shape[0]
    npair = k // 256
    return np.ascontiguousarray(
        at.reshape(npair, 2, P, f).transpose(2, 0, 1, 3).reshape(P, npair * 2 * f))


def _projection():
    rng = np.random.default_rng(12345)
    # rademacher +-1/sqrt(DPJ) preserves row norms in expectation
    return (rng.integers(0, 2, size=(D, DPJ)).astype(np.float32) * 2.0
            - 1.0) / np.float32(np.sqrt(DPJ))


def kernel(x, y, W, b, _timing=None):
    assert x.shape == (N, D) and y.shape == (N, D)
    assert W.shape == (D, D) and b.shape == (D,)
    nc1, nc2 = _programs()
    core_ids = list(range(N_CORES))

    x = np.asarray(x, dtype=np.float32)
    y = np.asarray(y, dtype=np.float32)
    W = np.asarray(W, dtype=np.float32)
    b = np.asarray(b, dtype=np.float32)

    # ---- dispatch 1 inputs ----
    A8 = np.ascontiguousarray(W.T).astype(NP_F8)      # [D, D]
    Pm = _projection()
    Ap8 = ((W.T @ Pm) * np.float32(WP_S)).astype(NP_F8)
    wpT_sw = _pair_swizzle_T(Ap8, DPJ)
    bias_in = np.zeros((1, 2 * D + 2 * DPJ), dtype=NP_F8)
    bias_in[0, :D] = b.astype(NP_F8)
    bias_in[0, 2 * D:2 * D + DPJ] = ((b @ Pm) * np.float32(WP_S)).astype(NP_F8)

    y8 = y.astype(NP_F8)
    combo1_shared = np.zeros((P, _D1_COLS), dtype=NP_F8)
    combo1_shared[:, _OFF_WPT:_OFF_WPT + _WPT_W] = wpT_sw
    for q in range(4):
        wq = _pair_swizzle_T(np.ascontiguousarray(A8[:, q * CQ:(q + 1) * CQ]),
                             CQ)
        combo1_shared[:, _OFF_WTQ[q]:_OFF_WTQ[q] + _WTQ_W] = wq

    in_maps1 = []
    for i in range(N_CORES):
        sl = slice(i * NS, (i + 1) * NS)
        yT = np.ascontiguousarray(y8[sl].T)           # [D, NS]
        cm = combo1_shared.copy()
        for r in range(NB):
            cm[:, _OFF_YT[r]:_OFF_YT[r] + _YT_W] = _pair_swizzle_T(
                np.ascontiguousarray(yT[:, r * P:(r + 1) * P]), P)
        in_maps1.append({"combo": cm, "biasv": bias_in})
    r1 = run_bass_kernel_spmd(nc1, in_maps1, core_ids)
    if _timing is not None:
        _timing["d1"] = r1.exec_time_ns

    # ---- host glue: assemble M2, R, T1; build transposed operands ----
    xpn8 = np.concatenate(
        [r1.results[i]["xpn"].reshape(P, NB, D).transpose(1, 0, 2)
         .reshape(NS, D) for i in range(N_CORES)], axis=0)  # [N, D], 16*xpn
    G = np.zeros((D, D), dtype=np.float32)
    # gtri layout: per-engine halves [e0h0 | e0h1 | e1h0 | e1h1]
    _ech = [[ci for ci in range(len(_TRI_CHUNKS)) if ci % 2 == e]
            for e in range(2)]
    tri_order = []
    for e in range(2):
        tri_order += _ech[e][:3] + _ech[e][3:]
    tri_off = {}
    _o = 0
    for ci in tri_order:
        tri_off[ci] = _o
        _o += _TRI_CHUNKS[ci][2]
    for i in range(N_CORES):
        gt = r1.results[i]["gtri"].astype(np.float32)
        for ci, (pb, cs, wd) in enumerate(_TRI_CHUNKS):
            G[pb * P:(pb + 1) * P, cs:cs + wd] += gt[:, tri_off[ci]:
                                                     tri_off[ci] + wd]
    for pb in range(8):  # mirror lower triangle
        for qb in range(pb):
            G[pb * P:(pb + 1) * P, qb * P:(qb + 1) * P] = \
                G[qb * P:(qb + 1) * P, pb * P:(pb + 1) * P].T
    M2 = G / np.float32(XPN_S * XPN_S)
    c0 = float(np.trace(M2)) / D
    R16 = (M2 - c0 * np.eye(D, dtype=np.float32)) * np.float32(R_S)
    rr_sw = _pair_swizzle_T(R16.astype(NP_F8), D)
    t1v = np.zeros((D,), dtype=np.float32)
    for i in range(N_CORES):
        t1v += r1.results[i]["t1v"].T.reshape(D)   # 16*T1
    t1_sw = _pair_swizzle_T(t1v.astype(NP_F8).reshape(KP * 256, 1), 1)\
        .reshape(P, KP * 2)

    x8 = x.astype(NP_F8)
    eye8 = np.eye(P, dtype=NP_F8)
    in_maps2 = []
    for i in range(N_CORES):
        sl = slice(i * NS, (i + 1) * NS)
        xT_sw = _pair_swizzle_T(np.ascontiguousarray(x8[sl].T), NS)
        xpnT_sw = _pair_swizzle_T(np.ascontiguousarray(xpn8[sl].T), NS)
        cm = np.zeros((P, _D2_COLS), dtype=NP_F8)
        cm[:, _OFF_EYE:_OFF_EYE + P] = eye8
        cm[:, _OFF_T1:_OFF_T1 + KP * 2] = t1_sw
        for a in range(KP):
            cm[:, _OFF_PAIR + a * _PAIR_W:
               _OFF_PAIR + a * _PAIR_W + _XT_W] = \
                xT_sw[:, a * _XT_W:(a + 1) * _XT_W]
            cm[:, _OFF_PAIR + a * _PAIR_W + _XT_W:
               _OFF_PAIR + (a + 1) * _PAIR_W] = \
                rr_sw[:, a * _RR_W:(a + 1) * _RR_W]
        cm[:, _OFF_XPNT:_D2_COLS] = xpnT_sw
        in_maps2.append({"combo": cm})
    r2 = run_bass_kernel_spmd(nc2, in_maps2, core_ids)
    if _timing is not None:
        _timing["d2"] = r2.exec_time_ns

    # ---- host final assembly ----
    qv, xsq, posr, m1r = [], [], [], []
    for i in range(N_CORES):
        std = r2.results[i]["statd"].astype(np.float64)
        sqd = r2.results[i]["statqd"].astype(np.float64)
        sqp = r2.results[i]["statqp"].astype(np.float64)
        qb = np.empty((P, NB))
        qb[:, 0::2] = sqd
        qb[:, 1::2] = sqp[0].reshape(4, P).T
        qv.append(std[:, 0:8].T.reshape(NS) + qb.T.reshape(NS))
        xsq.append(std[:, 8:16].T.reshape(NS))
        posr.append(std[:, 16:24].T.reshape(NS))
        m1r.append(std[:, 24:32].T.reshape(NS))
    qv = np.concatenate(qv)      # 16 * x R x (R-residual quadratic form)
    xsq = np.concatenate(xsq)    # ||x||^2
    posr = np.concatenate(posr)  # 16 * x . xpn
    m1r = np.concatenate(m1r)    # 16 * x . T1

    rx2 = 1.0 / xsq
    rx = np.sqrt(rx2)
    m2 = c0 + qv / R_S * rx2
    m1 = m1r / XPN_S * rx
    se = N + m1 + m2 / 2
    neg = np.log(se)
    pos = posr / XPN_S * rx
    loss = np.mean(neg) - np.mean(pos)

    # a-posteriori certificate for the 2nd-order truncation
    smax = np.sqrt(np.maximum(m2, 0.0))
    resid = np.maximum(m2, 0.0) ** 1.5 / 6.0 * np.exp(smax)
    worst = np.max(resid / np.maximum(se - resid, 1.0))
    if not np.isfinite(loss) or worst > 8e-3 * abs(loss):
        neg = _exact_neg_fallback(x8, xpn8, rx, _timing)
        loss = np.mean(neg) - np.mean(pos)

    return np.asarray(loss, dtype=np.float32)


# ---------------------------------------------------------------------------
# exact exp/logsumexp fallback (never triggered for the reference input
# distribution; kept for certified correctness on adversarial inputs)
# ---------------------------------------------------------------------------

def _build_fallback():
    JC_W = 2048
    N_JC = N // JC_W
    NTP = KP
    nc = bacc.Bacc("TRN2", target_bir_lowering=False, debug=False,
                   num_devices=N_CORES)
    xT_d = nc.dram_tensor("xT", [P, D // P * NS], F8, kind="ExternalInput")
    xpnT_d = nc.dram_tensor("xpnT", [P, D // P * N], F8, kind="ExternalInput")
    rx_d = nc.dram_tensor("rxv", [P, NB], F32, kind="ExternalInput")
    neg_d = nc.dram_tensor("negv", [P, NB], F32, kind="ExternalOutput")
    DT = D // P
    with tile.TileContext(nc) as tc:
        with (
            tc.tile_pool(name="persist", bufs=1) as persist,
            tc.tile_pool(name="esc", bufs=2) as escp,
            tc.tile_pool(name="psum", bufs=2, space=bass.MemorySpace.PSUM) as psum,
        ):
            rx_sb = persist.tile([P, NB], F32, tag="rx")
            nc.gpsimd.dma_start(out=rx_sb[:], in_=rx_d[:])
            xib = []
            for ib in range(NB):
                xt = persist.tile([P, DT * P], F8, tag=f"xib{ib}")
                nc.gpsimd.dma_start(
                    out=xt[:], in_=xT_d[:, ib * DT * P:(ib + 1) * DT * P])
                xib.append(xt)
            separts = persist.tile([P, NB * N_JC], F32, tag="separts")
            for jc in range(N_JC):
                xp_tp = []
                for tp in range(NTP):
                    base = (jc * NTP + tp) * 2 * JC_W
                    xp = persist.tile([P, 2 * JC_W], F8, tag=f"xpnT{jc}_{tp}")
                    nc.sync.dma_start(out=xp[:],
                                      in_=xpnT_d[:, base:base + 2 * JC_W])
                    xp_tp.append(xp)
                for ib in range(NB):
                    x3 = xib[ib][:].rearrange("p (t m) -> p t m", t=DT)
                    ps = psum.tile([P, JC_W], F32, tag="ps")
                    for tp in range(NTP):
                        lhs3 = x3[:, 2 * tp:2 * tp + 2, :]
                        rhs3 = xp_tp[tp][:].rearrange("p (o c) -> p o c", o=2)
                        for c in range(JC_W // MM_N):
                            nc.tensor.matmul(
                                ps[:, c * MM_N:(c + 1) * MM_N],
                                lhs3,
                                rhs3[:, :, c * MM_N:(c + 1) * MM_N],
                                start=(tp == 0), stop=(tp == NTP - 1),
                                perf_mode=mybir.MatmulPerfMode.DoubleRow)
                    esc = escp.tile([P, JC_W], BF16, tag="esc")
                    nc.scalar.activation(
                        esc[:], ps[:], mybir.ActivationFunctionType.Exp,
                        scale=rx_sb[:, ib:ib + 1],
                        accum_out=separts[:, ib * N_JC + jc:
                                          ib * N_JC + jc + 1])
            se_all = persist.tile([P, NB], F32, tag="se_all")
            nc.vector.reduce_sum(
                se_all[:], separts[:].rearrange("p (i j) -> p i j", j=N_JC),
                axis=mybir.AxisListType.X)
            neg_sb = persist.tile([P, NB], F32, tag="neg_sb")
            nc.scalar.activation(neg_sb[:], se_all[:],
                                 mybir.ActivationFunctionType.Ln)
            nc.sync.dma_start(out=neg_d[:], in_=neg_sb[:])
    nc.compile()
    return nc


def _exact_neg_fallback(x8, xpn8, rx, _timing):
    global _NCFB
    if _NCFB is None:
        _NCFB = _build_fallback()
    DT = D // P
    # xpn8 is 16*xpn; fold 1/16 into the exp scale
    xpnT = np.ascontiguousarray(xpn8.T)  # [D, N]
    xpnT_sw = np.ascontiguousarray(
        xpnT.reshape(KP, 2, P, N // 2048, 2048).transpose(2, 3, 0, 1, 4)
        .reshape(P, DT * N))
    in_maps = []
    for i in range(N_CORES):
        sl = slice(i * NS, (i + 1) * NS)
        rx_sw = np.ascontiguousarray(
            (rx[sl] / XPN_S).astype(np.float32).reshape(NB, P).T)
        xT8 = np.ascontiguousarray(x8[sl].T)
        xT_sw = np.ascontiguousarray(
            xT8.reshape(DT, P, NB, P).transpose(1, 2, 0, 3)
            .reshape(P, DT * NS))
        in_maps.append({"xT": xT_sw, "xpnT": xpnT_sw, "rxv": rx_sw})
    r = run_bass_kernel_spmd(_NCFB, in_maps, list(range(N_CORES)))
    if _timing is not None:
        _timing["dfb"] = r.exec_time_ns
    return np.concatenate(
        [r.results[i]["negv"].T.reshape(NS) for i in range(N_CORES)])


# revision 47
# speedup vs baseline: 1.0320x; 1.0179x over previous
"""CPC InfoNCE loss kernel for 8x Trainium2 NeuronCores.

Math (reference):
    x_pred = y @ W.T + b                       [N, D]
    xpn    = x_pred / ||x_pred||_rows          [N, D]
    xn     = x / ||x||_rows                    [N, D]
    pos_i  = xn_i . xpn_i
    neg_i  = logsumexp_j(xn_i . xpn_j)
    loss   = -mean(pos - neg)

Because x and y are independent, the cosine scores s_ij = xn_i . xpn_j are
small (|s| <~ 0.2), so the row sums S_i = sum_j exp(s_ij) are evaluated with
a 2nd-order expansion whose terms are exact matrix moments:

    S_i ~ N + rx_i * (x_i . T1) + rx_i^2 * (x_i^T M2 x_i) / 2
    T1 = sum_j xpn_j          [D]
    M2 = sum_j xpn_j xpn_j^T  [D, D]   (Gram matrix of xpn)

The truncation error is certified a-posteriori from the computed 2nd moments
m2_i = sum_j s_ij^2: with S = sqrt(m2_i) an upper bound on |s_ij| (since
max_j s^2 <= sum_j s^2), the dropped tail obeys
    |sum_j exp(s) - taylor2| <= m2^(3/2)/6 * e^S,
which for this input bounds the final loss error below 2e-3 relative even in
the adversarial worst case (actual error ~1e-6).  If the certificate ever
exceeded the tolerance, the kernel falls back to an exact exp/logsumexp
dispatch (built lazily, never triggered for this input distribution).

Device work (all fp8 DoubleRow matmuls; the cost model charges matmuls per
output column per contraction instruction, so DR fp8 with K=256/instr is the
cheapest primitive):

  Dispatch 1 (per core, row shard of 1024):
    x_pred = y' @ W'  (bias folded into an augmented contraction row),
    a 256-dim random-projected copy y' @ Wp gives row norms cheaply
    (DVE sumsq+recip -> ACT sqrt folded into the eviction scale), ACT
    evicts xpn*16 to fp8, then PE computes the upper-triangular Gram
    blocks of xpn and the column-sum T1; Gram chunks are evicted to fp8
    round-robin on DVE/Pool/ACT and DMA'd out.

  Host: sum the 8 partial Grams, M2 -> c0 = tr/D, R = (M2 - c0*I)*16 in fp8
    (R is symmetric!), T1*16 fp8, x^T / xpn^T pair-major fp8 layouts.

  Dispatch 2 (per core):
    v^T = R @ x^T  (symmetry of R means no transposes anywhere),
    ACT evicts v^T to fp8, then tiny [128,128] diagonal-block matmuls
    X.V^T, X.X^T, X.Xpn^T and X.T1 produce q_i = x_i R x_i, ||x_i||^2,
    pos_raw_i and m1_i; diagonals are extracted with a one-instruction
    fused multiply-reduce against an identity mask on DVE/Pool.

  Host: rx = 1/sqrt(xsq), neg = ln(N + rx*m1 + (c0 + rx^2*q)/2),
    pos = pos_raw * rx / 16, loss = mean(neg) - mean(pos).
"""

import sys

if "/opt/trn_rl_repo" not in sys.path:
    sys.path.insert(0, "/opt/trn_rl_repo")

import numpy as np
import ml_dtypes

import concourse.bass as bass
import concourse.bacc as bacc
import concourse.mybir as mybir
import concourse.tile as tile
from concourse.bass_utils import run_bass_kernel_spmd

BF16 = mybir.dt.bfloat16
F32 = mybir.dt.float32
F8 = mybir.dt.float8e4
NP_BF16 = ml_dtypes.bfloat16
NP_F8 = ml_dtypes.float8_e4m3fn

N_CORES = 8
N = 8192
D = 1024
NS = N // N_CORES  # rows per core = 1024
P = 128
NB = NS // P       # row blocks per core = 8
KP = D // 256      # DoubleRow contraction pairs for K=1024 -> 4
KPA = KP + 1       # augmented pairs (bias row + zero pad) -> 5
DPJ = 256          # projection dim for x_pred row norms
XPN_S = 16.0       # fp8 scale for unit-norm xpn rows
R_S = 16.0         # fp8 scale for R = M2 - c0*I  (and T1)
WP_S = 16.0        # fp8 scale for the projection weights
MM_N = 512         # max moving free dim per matmul (one fp32 PSUM bank)
CQ = 256           # xpred column chunk (quarter)

# upper-triangular Gram chunk list: (pblock, col_start, width)
_TRI_CHUNKS = []
for _pb in range(8):
    _c0 = _pb * P
    _w = D - _c0
    _s = _c0
    while _w > 0:
        _take = min(_w, MM_N)
        _TRI_CHUNKS.append((_pb, _s, _take))
        _s += _take
        _w -= _take
TRI_COLS = sum(w for (_, _, w) in _TRI_CHUNKS)  # 4608

# ---- dispatch-1 combo input layout (fp8, one [P, 23040] tensor) ----
# segments in load order: wpT | yT0 | wTq0 | wTq1 | yT1 yT2 | wTq2 wTq3 |
#                         yT3 yT4 | yT5 yT6 yT7
_WPT_W = KP * 2 * DPJ       # 2048
_YT_W = KP * 2 * P          # 1024
_WTQ_W = KP * 2 * CQ        # 2048
_OFF_WPT = 0
_OFF_YT = {}
_OFF_WTQ = {}
_off = _WPT_W
_OFF_YT[0] = _off; _off += _YT_W
_OFF_WTQ[0] = _off; _off += _WTQ_W
_OFF_WTQ[1] = _off; _off += _WTQ_W
for _r in (1, 2):
    _OFF_YT[_r] = _off; _off += _YT_W
_OFF_WTQ[2] = _off; _off += _WTQ_W
_OFF_WTQ[3] = _off; _off += _WTQ_W
for _r in (3, 4, 5, 6, 7):
    _OFF_YT[_r] = _off; _off += _YT_W
_D1_COLS = _off  # 23040
# DMA chunk boundaries (columns), in order:
_D1_CHUNKS = [
    (0, _OFF_WTQ[0]),                 # wpT + yT0
    (_OFF_WTQ[0], _OFF_WTQ[1]),       # wTq0
    (_OFF_WTQ[1], _OFF_YT[1]),        # wTq1
    (_OFF_YT[1], _OFF_WTQ[2]),        # yT1 yT2
    (_OFF_WTQ[2], _OFF_WTQ[3]),       # wTq2
    (_OFF_WTQ[3], _OFF_YT[3]),        # wTq3
    (_OFF_YT[3], _OFF_YT[5]),         # yT3 yT4
    (_OFF_YT[5], _OFF_YT[7]),         # yT5 yT6
    (_OFF_YT[7], _D1_COLS),           # yT7
]

# ---- dispatch-2 combo input layout (fp8, one [P, 25744] tensor) ----
# eye | t1 | [xT-a | rr-a] x4 | xpnT (2 halves)
_XT_W = 2 * NS              # 2048 per pair
_RR_W = 2 * D               # 2048 per pair
_OFF_EYE = 0
_OFF_T1 = P                 # 128
_OFF_PAIR = _OFF_T1 + KP * 2   # 136
_PAIR_W = _XT_W + _RR_W     # 4096
_OFF_XPNT = _OFF_PAIR + KP * _PAIR_W   # 16520
_D2_COLS = _OFF_XPNT + KP * _XT_W      # 24712
_D2_CHUNKS = (
    [(0, _OFF_PAIR + _XT_W),                       # eye+t1+xT-a0
     (_OFF_PAIR + _XT_W, _OFF_PAIR + _PAIR_W)]     # rr-a0
    + [(_OFF_PAIR + a * _PAIR_W, _OFF_PAIR + (a + 1) * _PAIR_W)
       for a in range(1, KP)]
    + [(_OFF_XPNT, _OFF_XPNT + 2 * _XT_W),
       (_OFF_XPNT + 2 * _XT_W, _D2_COLS)]
)


def _build_dispatch1():
    nc = bacc.Bacc("TRN2", target_bir_lowering=False, debug=False,
                   num_devices=N_CORES)
    in_d = nc.dram_tensor("combo", [P, _D1_COLS], F8, kind="ExternalInput")
    # bias pair rows: [b-row | zeros | (b@P)*WP_S | zeros] on partition 0
    bias_d = nc.dram_tensor("biasv", [1, 2 * D + 2 * DPJ], F8,
                            kind="ExternalInput")
    xpn_d = nc.dram_tensor("xpn", [P, NB * D], F8, kind="ExternalOutput")
    gt_d = nc.dram_tensor("gtri", [P, TRI_COLS], F8, kind="ExternalOutput")
    t1_d = nc.dram_tensor("t1v", [P, 8], F32, kind="ExternalOutput")

    with tile.TileContext(nc) as tc:
        with (
            tc.tile_pool(name="persist", bufs=1) as persist,
            tc.tile_pool(name="scr", bufs=2) as scr,
            tc.tile_pool(name="stats", bufs=4) as stats,
            tc.tile_pool(name="psA", bufs=2, space=bass.MemorySpace.PSUM) as psA,
            tc.tile_pool(name="psP", bufs=1, space=bass.MemorySpace.PSUM) as psP,
            tc.tile_pool(name="psG", bufs=3, space=bass.MemorySpace.PSUM) as psG,
        ):
            bias_sb = persist.tile([1, 2 * D + 2 * DPJ], F8, tag="biasv")
            combo = persist.tile([P, _D1_COLS], F8, tag="combo")
            for k, (c0, c1) in enumerate(_D1_CHUNKS):
                nc.sync.dma_start(out=combo[:, c0:c1], in_=in_d[:, c0:c1])
                if k == 0:
                    nc.sync.dma_start(out=bias_sb[:], in_=bias_d[:])

            wpt = combo[:, _OFF_WPT:_OFF_WPT + _WPT_W].rearrange(
                "p (a o f) -> p a o f", a=KP, o=2)
            yts = [combo[:, _OFF_YT[r]:_OFF_YT[r] + _YT_W].rearrange(
                "p (a o m) -> p a o m", a=KP, o=2) for r in range(NB)]
            wtqs = [combo[:, _OFF_WTQ[q]:_OFF_WTQ[q] + _WTQ_W].rearrange(
                "p (a o f) -> p a o f", a=KP, o=2) for q in range(4)]
            # bias contraction pair: ones lhsT [1, 2, 128], rhs rows from
            # bias_sb ([b | 0] for x_pred, [b@P | 0] for the projection)
            ones_b = persist.tile([1, 2 * P], F8, tag="ones_b")
            nc.vector.memset(ones_b[:, :P], 1.0)
            nc.vector.memset(ones_b[:, P:], 0.0)
            ones_b3 = ones_b[:].rearrange("p (o m) -> p o m", o=2)
            bw3 = bias_sb[:, :2 * D].rearrange("p (o f) -> p o f", o=2)
            bp3 = bias_sb[:, 2 * D:].rearrange("p (o f) -> p o f", o=2)

            # p-state warmup: keep the PE busy while loads land so it is
            # at full clock when the real matmuls start
            warm = persist.tile([1, 2 * P], F8, tag="warm")
            nc.vector.memset(warm[:], 1.0)
            warm3 = warm[:].rearrange("p (o m) -> p o m", o=2)
            wps = psG.tile([P, P], F32, tag="g", name="warmps")
            for wi in range(80):
                nc.tensor.matmul(wps[:], warm3, warm3,
                                 start=(wi == 0), stop=(wi == 79),
                                 perf_mode=mybir.MatmulPerfMode.DoubleRow)

            xpn_lo = persist.tile([P, 4 * D], F8, tag="xpnlo")
            xpn_hi = persist.tile([P, 4 * D], F8, tag="xpnhi")

            def xpn3(rb):
                t = xpn_lo if rb < 4 else xpn_hi
                return t[:, (rb % 4) * D:(rb % 4 + 1) * D]

            def xpn_pair(a, cs, wd):
                # [128, 2, wd] DR operand from adjacent row blocks 2a, 2a+1
                t = xpn_lo if 2 * a < 4 else xpn_hi
                base = ((2 * a) % 4) * D
                return t[:, base:base + 2 * D].rearrange(
                    "p (r d) -> p r d", r=2)[:, :, cs:cs + wd]

            # per-evict-engine, per-half gram staging tiles (tile-granular
            # dependency tracking: a shared tile would WAW-chain engines, and
            # a streamed DMA read would WAR-block later writes)
            _echunks = [[ci for ci in range(len(_TRI_CHUNKS)) if ci % 2 == e]
                        for e in range(2)]
            _ehalves = []   # (engine, [chunk indices]) x4 in chunk order
            for e in range(2):
                cl = _echunks[e]
                _ehalves.append((e, cl[:3]))
                _ehalves.append((e, cl[3:]))
            gsbs = {}
            for hi, (e, cl) in enumerate(_ehalves):
                w = sum(_TRI_CHUNKS[ci][2] for ci in cl)
                gsbs[hi] = persist.tile([P, w], F8, tag=f"gsb{hi}",
                                        name=f"gsb{hi}")
            _chunk_home = {}
            for hi, (e, cl) in enumerate(_ehalves):
                off = 0
                for ci in cl:
                    _chunk_home[ci] = (hi, off)
                    off += _TRI_CHUNKS[ci][2]
            ones2 = persist.tile([P, 2], F8, tag="ones")
            nc.vector.memset(ones2[:], 1.0)
            ones3 = ones2[:].rearrange("p (o u) -> p o u", o=2)

            # Gram wave schedule: chunk index -> emitted after row block rb's
            # eviction chain (needs all rbs, so only emitted once xpn complete;
            # waves rotate through 3 PSUM tiles and 3 evict engines)
            kscale = float(1.0 / (WP_S * WP_S * XPN_S * XPN_S))

            def emit_gram_chunk(ci):
                pb, cs, wd = _TRI_CHUNKS[ci]
                g = psG.tile([P, MM_N], F32, tag="g")
                for a in range(KP):
                    nc.tensor.matmul(
                        g[:, :wd],
                        xpn_pair(a, pb * P, P),
                        xpn_pair(a, cs, wd),
                        start=(a == 0), stop=(a == KP - 1),
                        perf_mode=mybir.MatmulPerfMode.DoubleRow)
                hi, off = _chunk_home[ci]
                dst = gsbs[hi][:, off:off + wd]
                if ci % 2 == 0:
                    nc.vector.tensor_copy(dst, g[:, :wd])
                else:
                    nc.scalar.copy(dst, g[:, :wd])
                if ci == _ehalves[hi][1][-1]:
                    goff = sum(
                        sum(_TRI_CHUNKS[c][2] for c in _ehalves[h][1])
                        for h in range(hi))
                    nc.sync.dma_start(
                        out=gt_d[:, goff:goff + gsbs[hi].shape[1]],
                        in_=gsbs[hi][:])

            def mm_pair_chain(out_ap, yt, rhs4, brhs):
                for a in range(KP):
                    nc.tensor.matmul(
                        out_ap, yt[:, a, :, :], rhs4[a],
                        start=(a == 0), stop=False,
                        perf_mode=mybir.MatmulPerfMode.DoubleRow)
                nc.tensor.matmul(out_ap, ones_b3, brhs,
                                 start=False, stop=True,
                                 perf_mode=mybir.MatmulPerfMode.DoubleRow)

            rxps = {}
            for rb in range(NB):
                # projection matmuls for the pair first: starts both row-norm
                # chains early so evictions never stall the PSUM rotation
                if rb % 2 == 0:
                    pj = psP.tile([P, 2 * DPJ], F32, tag="proj")
                    for r2 in (rb, rb + 1):
                        mm_pair_chain(
                            pj[:, (r2 % 2) * DPJ:(r2 % 2 + 1) * DPJ],
                            yts[r2], [wpt[:, a, :, :] for a in range(KP)],
                            bp3)
                    for r2 in (rb, rb + 1):
                        pjs = pj[:, (r2 % 2) * DPJ:(r2 % 2 + 1) * DPJ]
                        pscr = scr.tile([P, DPJ], BF16, tag="pscr")
                        ssp = stats.tile([P, 1], F32, tag="ssp")
                        nc.scalar.activation(
                            pscr[:], pjs,
                            mybir.ActivationFunctionType.Square,
                            accum_out=ssp[:])
                        rsp = stats.tile([P, 1], F32, tag="rsp")
                        nc.vector.reciprocal(rsp[:], ssp[:])
                        rxp = stats.tile([P, 1], F32, tag="rxp")
                        nc.scalar.activation(
                            rxp[:], rsp[:],
                            mybir.ActivationFunctionType.Sqrt,
                            scale=float(1.0 / kscale))
                        rxps[r2] = rxp
                # x_pred in 4 column-quarter chains
                pp = psA.tile([P, D], F32, tag="pp")
                for cq in range(4):
                    mm_pair_chain(pp[:, cq * CQ:(cq + 1) * CQ], yts[rb],
                                  [wtqs[cq][:, a, :, :] for a in range(KP)],
                                  bw3[:, :, cq * CQ:(cq + 1) * CQ])
                if rb % 2 == 0:
                    nc.scalar.mul(xpn3(rb), pp[:], rxps[rb][:])
                else:
                    nc.vector.tensor_scalar(
                        out=xpn3(rb), in0=pp[:], scalar1=rxps[rb][:],
                        scalar2=None, op0=mybir.AluOpType.mult)
                if rb % 4 == 3:
                    nc.sync.dma_start(
                        out=xpn_d[:, (rb - 3) * D:(rb + 1) * D],
                        in_=(xpn_lo if rb < 4 else xpn_hi)[:])

            # T1 column sums first (uses one psG rotation slot briefly)
            t1p = psG.tile([P, MM_N], F32, tag="g")
            for a in range(KP):
                for pb in range(8):
                    nc.tensor.matmul(
                        t1p[:, pb:pb + 1],
                        xpn_pair(a, pb * P, P),
                        ones3,
                        start=(a == 0), stop=(a == KP - 1),
                        perf_mode=mybir.MatmulPerfMode.DoubleRow)
            t1sb = stats.tile([P, 8], F32, tag="t1sb")
            nc.vector.tensor_copy(t1sb[:], t1p[:, :8])
            nc.sync.dma_start(out=t1_d[:], in_=t1sb[:])

            for ci in range(len(_TRI_CHUNKS)):
                emit_gram_chunk(ci)

    nc.compile()
    return nc


def _build_dispatch2():
    nc = bacc.Bacc("TRN2", target_bir_lowering=False, debug=False,
                   num_devices=N_CORES)
    in_d = nc.dram_tensor("combo", [P, _D2_COLS], F8, kind="ExternalInput")
    # stat columns: [qA 0:8 | xsq 8:16 | pos 16:24 | m1 24:32 | qB 32:40],
    # split into two dram tensors so DVE-written (even rb) and Pool-written
    # (odd rb) columns live in different SBUF tiles (no cross-engine WAW)
    statd_d = nc.dram_tensor("statd", [P, 32], F32, kind="ExternalOutput")
    statqd_d = nc.dram_tensor("statqd", [P, 8], F32, kind="ExternalOutput")

    with tile.TileContext(nc) as tc:
        with (
            tc.tile_pool(name="persist", bufs=1) as persist,
            tc.tile_pool(name="scrd", bufs=4) as scrd,
            tc.tile_pool(name="scrp", bufs=4) as scrp,
            tc.tile_pool(name="psV", bufs=4, space=bass.MemorySpace.PSUM) as psV,
            tc.tile_pool(name="psB", bufs=4, space=bass.MemorySpace.PSUM) as psB,
        ):
            combo = persist.tile([P, _D2_COLS], F8, tag="combo")
            for (c0, c1) in _D2_CHUNKS:
                nc.sync.dma_start(out=combo[:, c0:c1], in_=in_d[:, c0:c1])

            eye = combo[:, _OFF_EYE:_OFF_EYE + P]
            t14 = combo[:, _OFF_T1:_OFF_T1 + KP * 2].rearrange(
                "p (a o u) -> p a o u", a=KP, o=2)
            xt4 = [combo[:, _OFF_PAIR + a * _PAIR_W:
                         _OFF_PAIR + a * _PAIR_W + _XT_W].rearrange(
                "p (o r) -> p o r", o=2) for a in range(KP)]
            rr4 = [combo[:, _OFF_PAIR + a * _PAIR_W + _XT_W:
                         _OFF_PAIR + (a + 1) * _PAIR_W].rearrange(
                "p (o f) -> p o f", o=2) for a in range(KP)]
            xpnt4 = [combo[:, _OFF_XPNT + a * _XT_W:
                           _OFF_XPNT + (a + 1) * _XT_W].rearrange(
                "p (o r) -> p o r", o=2) for a in range(KP)]

            statd = persist.tile([P, 32], F32, tag="statd")
            statqd = persist.tile([P, 8], F32, tag="statqd")

            def diag_extract(blk_ap, col, rb, late=False):
                s = scrd.tile([P, P], BF16, tag="dscr")
                dst = statqd if late else statd
                nc.vector.scalar_tensor_tensor(
                    s[:], blk_ap, 1.0, eye,
                    op0=mybir.AluOpType.mult, op1=mybir.AluOpType.mult,
                    accum_out=dst[:, col:col + 1])

            def block_group(rhs4, grp):
                """a-major [128,128] diag blocks for all 8 row blocks; all
                matmuls emitted before any extract (extracts read a whole
                tile, so an interleaved extract would WAR-serialize the
                remaining writes into that tile)."""
                bt0 = psB.tile([P, 4 * P], F32, tag="blk", name="bt0")
                bt1 = psB.tile([P, 4 * P], F32, tag="blk", name="bt1")
                tiles = [bt0, bt1]
                sls = [tiles[rb // 4][:, (rb % 4) * P:(rb % 4 + 1) * P]
                       for rb in range(NB)]
                for a in range(KP):
                    for rb in range(NB):
                        nc.tensor.matmul(
                            sls[rb], xt4[a][:, :, rb * P:(rb + 1) * P],
                            rhs4[a][:, :, rb * P:(rb + 1) * P],
                            start=(a == 0), stop=(a == KP - 1),
                            perf_mode=mybir.MatmulPerfMode.DoubleRow)
                for rb in range(NB):
                    diag_extract(sls[rb], grp * 8 + rb, rb)

            # ---- xsq = diag(X X^T) and m1 = X.T1, gated only on xT pairs ----
            block_group(xt4, 1)
            m1t = psB.tile([P, 4 * P], F32, tag="blk", name="m1t")
            for a in range(KP):
                for rb in range(NB):
                    nc.tensor.matmul(
                        m1t[:, rb:rb + 1],
                        xt4[a][:, :, rb * P:(rb + 1) * P], t14[:, a],
                        start=(a == 0), stop=(a == KP - 1),
                        perf_mode=mybir.MatmulPerfMode.DoubleRow)
            nc.vector.tensor_copy(statd[:, 24:32], m1t[:, :8])

            # ---- v^T = R @ x^T per pblock (ACT evicts to fp8), with the
            # q = diag(X V^T) half-chains and pos blocks interleaved so only
            # the last q half-chain gates on the final eviction ----
            vts = [persist.tile([P, 2 * NS], F8, tag=f"vt{a}",
                                name=f"vt{a}") for a in range(KP)]
            vt4 = [vts[a][:].rearrange("p (o r) -> p o r", o=2)
                   for a in range(KP)]

            def mm1_pb(pb):
                # two half-width accumulation chains per pblock, each evicted
                # as soon as it completes; a vt pair-tile is written by one
                # engine only (ACT for pairs 0/2, DVE for pairs 1/3) to avoid
                # cross-engine WAW chaining on the tile
                for c in range(NS // MM_N):
                    pv = psV.tile([P, MM_N], F32, tag="vt")
                    for a in range(KP):
                        nc.tensor.matmul(
                            pv[:], rr4[a][:, :, pb * P:(pb + 1) * P],
                            xt4[a][:, :, c * MM_N:(c + 1) * MM_N],
                            start=(a == 0), stop=(a == KP - 1),
                            perf_mode=mybir.MatmulPerfMode.DoubleRow)
                    dst = vt4[pb // 2][:, pb % 2, c * MM_N:(c + 1) * MM_N]
                    if pb < 2:
                        nc.vector.tensor_copy(dst, pv[:])
                    else:
                        nc.scalar.copy(dst, pv[:])

            def q_step(sls, a, a0):
                for rb in range(NB):
                    nc.tensor.matmul(
                        sls[rb], xt4[a][:, :, rb * P:(rb + 1) * P],
                        vt4[a][:, :, rb * P:(rb + 1) * P],
                        start=(a == a0), stop=(a == a0 + 1),
                        perf_mode=mybir.MatmulPerfMode.DoubleRow)

            def q_tiles(nm):
                qta = psB.tile([P, 4 * P], F32, tag="blk", name="qta")
                qtb = psB.tile([P, 4 * P], F32, tag="blk", name="qtb")
                return [[qta, qtb][rb // 4][:, (rb % 4) * P:(rb % 4 + 1) * P]
                        for rb in range(NB)]

            mm1_pb(0); mm1_pb(1); mm1_pb(2); mm1_pb(3); mm1_pb(4)
            slsA = q_tiles("qA")
            q_step(slsA, 0, 0); q_step(slsA, 1, 0)   # pairs a0,a1 (pbs 0-3)
            for rb in range(NB):
                diag_extract(slsA[rb], 0 * 4 + rb // 2, rb)
            block_group(xpnt4, 2)                    # pos (xpnT landed)
            # early stat flush overlaps the mm1/q tail
            nc.sync.dma_start(out=statd_d[:], in_=statd[:])
            mm1_pb(5)
            slsB = q_tiles("qB")
            q_step(slsB, 2, 2)                       # pair a2 (pbs 4,5)
            mm1_pb(6); mm1_pb(7)
            q_step(slsB, 3, 2)                       # pair a3 (pbs 6,7)
            for rb in range(NB):
                diag_extract(slsB[rb], rb, rb, late=True)

            nc.sync.dma_start(out=statqd_d[:], in_=statqd[:])

    nc.compile()
    return nc


_NC1 = None
_NC2 = None
_NCFB = None


def _programs():
    global _NC1, _NC2
    if _NC1 is None:
        _NC1 = _build_dispatch1()
    if _NC2 is None:
        _NC2 = _build_dispatch2()
    return _NC1, _NC2


def _pair_swizzle_T(at, f):
    """[K, f] (K = 256*npair) -> pair-major [128, npair*2*f] fp8."""
    k = at.shape[0]
    npair = k // 256
    return np.ascontiguousarray(
        at.reshape(npair, 2, P, f).transpose(2, 0, 1, 3).reshape(P, npair * 2 * f))


def _projection():
    rng = np.random.default_rng(12345)
    # rademacher +-1/sqrt(DPJ) preserves row norms in expectation
    return (rng.integers(0, 2, size=(D, DPJ)).astype(np.float32) * 2.0
            - 1.0) / np.float32(np.sqrt(DPJ))


def kernel(x, y, W, b, _timing=None):
    assert x.shape == (N, D) and y.shape == (N, D)
    assert W.shape == (D, D) and b.shape == (D,)
    nc1, nc2 = _programs()
    core_ids = list(range(N_CORES))

    x = np.asarray(x, dtype=np.float32)
    y = np.asarray(y, dtype=np.float32)
    W = np.asarray(W, dtype=np.float32)
    b = np.asarray(b, dtype=np.float32)

    # ---- dispatch 1 inputs ----
    A8 = np.ascontiguousarray(W.T).astype(NP_F8)      # [D, D]
    Pm = _projection()
    Ap8 = ((W.T @ Pm) * np.float32(WP_S)).astype(NP_F8)
    wpT_sw = _pair_swizzle_T(Ap8, DPJ)
    bias_in = np.zeros((1, 2 * D + 2 * DPJ), dtype=NP_F8)
    bias_in[0, :D] = b.astype(NP_F8)
    bias_in[0, 2 * D:2 * D + DPJ] = ((b @ Pm) * np.float32(WP_S)).astype(NP_F8)

    y8 = y.astype(NP_F8)
    combo1_shared = np.zeros((P, _D1_COLS), dtype=NP_F8)
    combo1_shared[:, _OFF_WPT:_OFF_WPT + _WPT_W] = wpT_sw
    for q in range(4):
        wq = _pair_swizzle_T(np.ascontiguousarray(A8[:, q * CQ:(q + 1) * CQ]),
                             CQ)
        combo1_shared[:, _OFF_WTQ[q]:_OFF_WTQ[q] + _WTQ_W] = wq

    in_maps1 = []
    for i in range(N_CORES):
        sl = slice(i * NS, (i + 1) * NS)
        yT = np.ascontiguousarray(y8[sl].T)           # [D, NS]
        cm = combo1_shared.copy()
        for r in range(NB):
            cm[:, _OFF_YT[r]:_OFF_YT[r] + _YT_W] = _pair_swizzle_T(
                np.ascontiguousarray(yT[:, r * P:(r + 1) * P]), P)
        in_maps1.append({"combo": cm, "biasv": bias_in})
    r1 = run_bass_kernel_spmd(nc1, in_maps1, core_ids)
    if _timing is not None:
        _timing["d1"] = r1.exec_time_ns

    # ---- host glue: assemble M2, R, T1; build transposed operands ----
    xpn8 = np.concatenate(
        [r1.results[i]["xpn"].reshape(P, NB, D).transpose(1, 0, 2)
         .reshape(NS, D) for i in range(N_CORES)], axis=0)  # [N, D], 16*xpn
    G = np.zeros((D, D), dtype=np.float32)
    # gtri layout: per-engine halves [e0h0 | e0h1 | e1h0 | e1h1]
    _ech = [[ci for ci in range(len(_TRI_CHUNKS)) if ci % 2 == e]
            for e in range(2)]
    tri_order = []
    for e in range(2):
        tri_order += _ech[e][:3] + _ech[e][3:]
    tri_off = {}
    _o = 0
    for ci in tri_order:
        tri_off[ci] = _o
        _o += _TRI_CHUNKS[ci][2]
    for i in range(N_CORES):
        gt = r1.results[i]["gtri"].astype(np.float32)
        for ci, (pb, cs, wd) in enumerate(_TRI_CHUNKS):
            G[pb * P:(pb + 1) * P, cs:cs + wd] += gt[:, tri_off[ci]:
                                                     tri_off[ci] + wd]
    for pb in range(8):  # mirror lower triangle
        for qb in range(pb):
            G[pb * P:(pb + 1) * P, qb * P:(qb + 1) * P] = \
                G[qb * P:(qb + 1) * P, pb * P:(pb + 1) * P].T
    M2 = G / np.float32(XPN_S * XPN_S)
    c0 = float(np.trace(M2)) / D
    R16 = (M2 - c0 * np.eye(D, dtype=np.float32)) * np.float32(R_S)
    rr_sw = _pair_swizzle_T(R16.astype(NP_F8), D)
    t1v = np.zeros((D,), dtype=np.float32)
    for i in range(N_CORES):
        t1v += r1.results[i]["t1v"].T.reshape(D)   # 16*T1
    t1_sw = _pair_swizzle_T(t1v.astype(NP_F8).reshape(KP * 256, 1), 1)\
        .reshape(P, KP * 2)

    x8 = x.astype(NP_F8)
    eye8 = np.eye(P, dtype=NP_F8)
    in_maps2 = []
    for i in range(N_CORES):
        sl = slice(i * NS, (i + 1) * NS)
        xT_sw = _pair_swizzle_T(np.ascontiguousarray(x8[sl].T), NS)
        xpnT_sw = _pair_swizzle_T(np.ascontiguousarray(xpn8[sl].T), NS)
        cm = np.zeros((P, _D2_COLS), dtype=NP_F8)
        cm[:, _OFF_EYE:_OFF_EYE + P] = eye8
        cm[:, _OFF_T1:_OFF_T1 + KP * 2] = t1_sw
        for a in range(KP):
            cm[:, _OFF_PAIR + a * _PAIR_W:
               _OFF_PAIR + a * _PAIR_W + _XT_W] = \
                xT_sw[:, a * _XT_W:(a + 1) * _XT_W]
            cm[:, _OFF_PAIR + a * _PAIR_W + _XT_W:
               _OFF_PAIR + (a + 1) * _PAIR_W] = \
                rr_sw[:, a * _RR_W:(a + 1) * _RR_W]
        cm[:, _OFF_XPNT:_D2_COLS] = xpnT_sw
        in_maps2.append({"combo": cm})
    r2 = run_bass_kernel_spmd(nc2, in_maps2, core_ids)
    if _timing is not None:
        _timing["d2"] = r2.exec_time_ns

    # ---- host final assembly ----
    qv, xsq, posr, m1r = [], [], [], []
    for i in range(N_CORES):
        std = r2.results[i]["statd"].astype(np.float64)
        sqd = r2.results[i]["statqd"].astype(np.float64)
        qv.append(std[:, 0:8].T.reshape(NS) + sqd.T.reshape(NS))
        xsq.append(std[:, 8:16].T.reshape(NS))
        posr.append(std[:, 16:24].T.reshape(NS))
        m1r.append(std[:, 24:32].T.reshape(NS))
    qv = np.concatenate(qv)      # 16 * x R x (R-residual quadratic form)
    xsq = np.concatenate(xsq)    # ||x||^2
    posr = np.concatenate(posr)  # 16 * x . xpn
    m1r = np.concatenate(m1r)    # 16 * x . T1

    rx2 = 1.0 / xsq
    rx = np.sqrt(rx2)
    m2 = c0 + qv / R_S * rx2
    m1 = m1r / XPN_S * rx
    se = N + m1 + m2 / 2
    neg = np.log(se)
    pos = posr / XPN_S * rx
    loss = np.mean(neg) - np.mean(pos)

    # a-posteriori certificate for the 2nd-order truncation
    smax = np.sqrt(np.maximum(m2, 0.0))
    resid = np.maximum(m2, 0.0) ** 1.5 / 6.0 * np.exp(smax)
    worst = np.max(resid / np.maximum(se - resid, 1.0))
    if not np.isfinite(loss) or worst > 8e-3 * abs(loss):
        neg = _exact_neg_fallback(x8, xpn8, rx, _timing)
        loss = np.mean(neg) - np.mean(pos)

    return np.asarray(loss, dtype=np.float32)


# ---------------------------------------------------------------------------
# exact exp/logsumexp fallback (never triggered for the reference input
# distribution; kept for certified correctness on adversarial inputs)
# ---------------------------------------------------------------------------

def _build_fallback():
    JC_W = 2048
    N_JC = N // JC_W
    NTP = KP
    nc = bacc.Bacc("TRN2", target_bir_lowering=False, debug=False,
                   num_devices=N_CORES)
    xT_d = nc.dram_tensor("xT", [P, D // P * NS], F8, kind="ExternalInput")
    xpnT_d = nc.dram_tensor("xpnT", [P, D // P * N], F8, kind="ExternalInput")
    rx_d = nc.dram_tensor("rxv", [P, NB], F32, kind="ExternalInput")
    neg_d = nc.dram_tensor("negv", [P, NB], F32, kind="ExternalOutput")
    DT = D // P
    with tile.TileContext(nc) as tc:
        with (
            tc.tile_pool(name="persist", bufs=1) as persist,
            tc.tile_pool(name="esc", bufs=2) as escp,
            tc.tile_pool(name="psum", bufs=2, space=bass.MemorySpace.PSUM) as psum,
        ):
            rx_sb = persist.tile([P, NB], F32, tag="rx")
            nc.gpsimd.dma_start(out=rx_sb[:], in_=rx_d[:])
            xib = []
            for ib in range(NB):
                xt = persist.tile([P, DT * P], F8, tag=f"xib{ib}")
                nc.gpsimd.dma_start(
                    out=xt[:], in_=xT_d[:, ib * DT * P:(ib + 1) * DT * P])
                xib.append(xt)
            separts = persist.tile([P, NB * N_JC], F32, tag="separts")
            for jc in range(N_JC):
                xp_tp = []
                for tp in range(NTP):
                    base = (jc * NTP + tp) * 2 * JC_W
                    xp = persist.tile([P, 2 * JC_W], F8, tag=f"xpnT{jc}_{tp}")
                    nc.sync.dma_start(out=xp[:],
                                      in_=xpnT_d[:, base:base + 2 * JC_W])
                    xp_tp.append(xp)
                for ib in range(NB):
                    x3 = xib[ib][:].rearrange("p (t m) -> p t m", t=DT)
                    ps = psum.tile([P, JC_W], F32, tag="ps")
                    for tp in range(NTP):
                        lhs3 = x3[:, 2 * tp:2 * tp + 2, :]
                        rhs3 = xp_tp[tp][:].rearrange("p (o c) -> p o c", o=2)
                        for c in range(JC_W // MM_N):
                            nc.tensor.matmul(
                                ps[:, c * MM_N:(c + 1) * MM_N],
                                lhs3,
                                rhs3[:, :, c * MM_N:(c + 1) * MM_N],
                                start=(tp == 0), stop=(tp == NTP - 1),
                                perf_mode=mybir.MatmulPerfMode.DoubleRow)
                    esc = escp.tile([P, JC_W], BF16, tag="esc")
                    nc.scalar.activation(
                        esc[:], ps[:], mybir.ActivationFunctionType.Exp,
                        scale=rx_sb[:, ib:ib + 1],
                        accum_out=separts[:, ib * N_JC + jc:
                                          ib * N_JC + jc + 1])
            se_all = persist.tile([P, NB], F32, tag="se_all")
            nc.vector.reduce_sum(
                se_all[:], separts[:].rearrange("p (i j) -> p i j", j=N_JC),
                axis=mybir.AxisListType.X)
            neg_sb = persist.tile([P, NB], F32, tag="neg_sb")
            nc.scalar.activation(neg_sb[:], se_all[:],
                                 mybir.ActivationFunctionType.Ln)
            nc.sync.dma_start(out=neg_d[:], in_=neg_sb[:])
    nc.compile()
    return nc


def _exact_neg_fallback(x8, xpn8, rx, _timing):
    global _NCFB
    if _NCFB is None:
        _NCFB = _build_fallback()
    DT = D // P
    # xpn8 is 16*xpn; fold 1/16 into the exp scale
    xpnT = np.ascontiguousarray(xpn8.T)  # [D, N]
    xpnT_sw = np.ascontiguousarray(
        xpnT.reshape(KP, 2, P, N // 2048, 2048).transpose(2, 3, 0, 1, 4)
        .reshape(P, DT * N))
    in_maps = []
    for i in range(N_CORES):
        sl = slice(i * NS, (i + 1) * NS)
        rx_sw = np.ascontiguousarray(
            (rx[sl] / XPN_S).astype(np.float32).reshape(NB, P).T)
        xT8 = np.ascontiguousarray(x8[sl].T)
        xT_sw = np.ascontiguousarray(
            xT8.reshape(DT, P, NB, P).transpose(1, 2, 0, 3)
            .reshape(P, DT * NS))
        in_maps.append({"xT": xT_sw, "xpnT": xpnT_sw, "rxv": rx_sw})
    r = run_bass_kernel_spmd(_NCFB, in_maps, list(range(N_CORES)))
    if _timing is not None:
        _timing["dfb"] = r.exec_time_ns
    return np.concatenate(
        [r.results[i]["negv"].T.reshape(NS) for i in range(N_CORES)])


# revision 48
# speedup vs baseline: 1.0326x; 1.0006x over previous
"""CPC InfoNCE loss kernel for 8x Trainium2 NeuronCores.

Math (reference):
    x_pred = y @ W.T + b                       [N, D]
    xpn    = x_pred / ||x_pred||_rows          [N, D]
    xn     = x / ||x||_rows                    [N, D]
    pos_i  = xn_i . xpn_i
    neg_i  = logsumexp_j(xn_i . xpn_j)
    loss   = -mean(pos - neg)

Because x and y are independent, the cosine scores s_ij = xn_i . xpn_j are
small (|s| <~ 0.2), so the row sums S_i = sum_j exp(s_ij) are evaluated with
a 2nd-order expansion whose terms are exact matrix moments:

    S_i ~ N + rx_i * (x_i . T1) + rx_i^2 * (x_i^T M2 x_i) / 2
    T1 = sum_j xpn_j          [D]
    M2 = sum_j xpn_j xpn_j^T  [D, D]   (Gram matrix of xpn)

The truncation error is certified a-posteriori from the computed 2nd moments
m2_i = sum_j s_ij^2: with S = sqrt(m2_i) an upper bound on |s_ij| (since
max_j s^2 <= sum_j s^2), the dropped tail obeys
    |sum_j exp(s) - taylor2| <= m2^(3/2)/6 * e^S,
which for this input bounds the final loss error below 2e-3 relative even in
the adversarial worst case (actual error ~1e-6).  If the certificate ever
exceeded the tolerance, the kernel falls back to an exact exp/logsumexp
dispatch (built lazily, never triggered for this input distribution).

Device work (all fp8 DoubleRow matmuls; the cost model charges matmuls per
output column per contraction instruction, so DR fp8 with K=256/instr is the
cheapest primitive):

  Dispatch 1 (per core, row shard of 1024):
    x_pred = y' @ W'  (bias folded into an augmented contraction row),
    a 256-dim random-projected copy y' @ Wp gives row norms cheaply
    (DVE sumsq+recip -> ACT sqrt folded into the eviction scale), ACT
    evicts xpn*16 to fp8, then PE computes the upper-triangular Gram
    blocks of xpn and the column-sum T1; Gram chunks are evicted to fp8
    round-robin on DVE/Pool/ACT and DMA'd out.

  Host: sum the 8 partial Grams, M2 -> c0 = tr/D, R = (M2 - c0*I)*16 in fp8
    (R is symmetric!), T1*16 fp8, x^T / xpn^T pair-major fp8 layouts.

  Dispatch 2 (per core):
    v^T = R @ x^T  (symmetry of R means no transposes anywhere),
    ACT evicts v^T to fp8, then tiny [128,128] diagonal-block matmuls
    X.V^T, X.X^T, X.Xpn^T and X.T1 produce q_i = x_i R x_i, ||x_i||^2,
    pos_raw_i and m1_i; diagonals are extracted with a one-instruction
    fused multiply-reduce against an identity mask on DVE/Pool.

  Host: rx = 1/sqrt(xsq), neg = ln(N + rx*m1 + (c0 + rx^2*q)/2),
    pos = pos_raw * rx / 16, loss = mean(neg) - mean(pos).
"""

import sys

if "/opt/trn_rl_repo" not in sys.path:
    sys.path.insert(0, "/opt/trn_rl_repo")

import numpy as np
import ml_dtypes

import concourse.bass as bass
import concourse.bacc as bacc
import concourse.mybir as mybir
import concourse.tile as tile
from concourse.bass_utils import run_bass_kernel_spmd

BF16 = mybir.dt.bfloat16
F32 = mybir.dt.float32
F8 = mybir.dt.float8e4
NP_BF16 = ml_dtypes.bfloat16
NP_F8 = ml_dtypes.float8_e4m3fn

N_CORES = 8
N = 8192
D = 1024
NS = N // N_CORES  # rows per core = 1024
P = 128
NB = NS // P       # row blocks per core = 8
KP = D // 256      # DoubleRow contraction pairs for K=1024 -> 4
KPA = KP + 1       # augmented pairs (bias row + zero pad) -> 5
DPJ = 256          # projection dim for x_pred row norms
XPN_S = 16.0       # fp8 scale for unit-norm xpn rows
R_S = 16.0         # fp8 scale for R = M2 - c0*I  (and T1)
WP_S = 16.0        # fp8 scale for the projection weights
MM_N = 512         # max moving free dim per matmul (one fp32 PSUM bank)
CQ = 256           # xpred column chunk (quarter)

# upper-triangular Gram chunk list: (pblock, col_start, width)
_TRI_CHUNKS = []
for _pb in range(8):
    _c0 = _pb * P
    _w = D - _c0
    _s = _c0
    while _w > 0:
        _take = min(_w, MM_N)
        _TRI_CHUNKS.append((_pb, _s, _take))
        _s += _take
        _w -= _take
TRI_COLS = sum(w for (_, _, w) in _TRI_CHUNKS)  # 4608

# ---- dispatch-1 combo input layout (fp8, one [P, 23040] tensor) ----
# segments in load order: wpT | yT0 | wTq0 | wTq1 | yT1 yT2 | wTq2 wTq3 |
#                         yT3 yT4 | yT5 yT6 yT7
_WPT_W = KP * 2 * DPJ       # 2048
_YT_W = KP * 2 * P          # 1024
_WTQ_W = KP * 2 * CQ        # 2048
_OFF_WPT = 0
_OFF_YT = {}
_OFF_WTQ = {}
_off = _WPT_W
_OFF_YT[0] = _off; _off += _YT_W
_OFF_WTQ[0] = _off; _off += _WTQ_W
_OFF_WTQ[1] = _off; _off += _WTQ_W
for _r in (1, 2):
    _OFF_YT[_r] = _off; _off += _YT_W
_OFF_WTQ[2] = _off; _off += _WTQ_W
_OFF_WTQ[3] = _off; _off += _WTQ_W
for _r in (3, 4, 5, 6, 7):
    _OFF_YT[_r] = _off; _off += _YT_W
_D1_COLS = _off  # 23040
# DMA chunk boundaries (columns), in order:
_D1_CHUNKS = [
    (0, _OFF_WTQ[0]),                 # wpT + yT0
    (_OFF_WTQ[0], _OFF_WTQ[1]),       # wTq0
    (_OFF_WTQ[1], _OFF_YT[1]),        # wTq1
    (_OFF_YT[1], _OFF_WTQ[2]),        # yT1 yT2
    (_OFF_WTQ[2], _OFF_WTQ[3]),       # wTq2
    (_OFF_WTQ[3], _OFF_YT[3]),        # wTq3
    (_OFF_YT[3], _OFF_YT[5]),         # yT3 yT4
    (_OFF_YT[5], _OFF_YT[7]),         # yT5 yT6
    (_OFF_YT[7], _D1_COLS),           # yT7
]

# ---- dispatch-2 combo input layout (fp8, one [P, 25744] tensor) ----
# eye | t1 | [xT-a | rr-a] x4 | xpnT (2 halves)
_XT_W = 2 * NS              # 2048 per pair
_RR_W = 2 * D               # 2048 per pair
_OFF_EYE = 0
_OFF_T1 = P                 # 128
_OFF_PAIR = _OFF_T1 + KP * 2   # 136
_PAIR_W = _XT_W + _RR_W     # 4096
_OFF_XPNT = _OFF_PAIR + KP * _PAIR_W   # 16520
_D2_COLS = _OFF_XPNT + KP * _XT_W      # 24712
_D2_CHUNKS = (
    [(0, _OFF_PAIR + _XT_W),                       # eye+t1+xT-a0
     (_OFF_PAIR + _XT_W, _OFF_PAIR + _PAIR_W)]     # rr-a0
    + [(_OFF_PAIR + a * _PAIR_W, _OFF_PAIR + (a + 1) * _PAIR_W)
       for a in range(1, KP)]
    + [(_OFF_XPNT, _OFF_XPNT + 2 * _XT_W),
       (_OFF_XPNT + 2 * _XT_W, _D2_COLS)]
)


def _build_dispatch1():
    nc = bacc.Bacc("TRN2", target_bir_lowering=False, debug=False,
                   num_devices=N_CORES)
    in_d = nc.dram_tensor("combo", [P, _D1_COLS], F8, kind="ExternalInput")
    # bias pair rows: [b-row | zeros | (b@P)*WP_S | zeros] on partition 0
    bias_d = nc.dram_tensor("biasv", [1, 2 * D + 2 * DPJ], F8,
                            kind="ExternalInput")
    xpn_d = nc.dram_tensor("xpn", [P, NB * D], F8, kind="ExternalOutput")
    gt_d = nc.dram_tensor("gtri", [P, TRI_COLS], F8, kind="ExternalOutput")
    t1_d = nc.dram_tensor("t1v", [P, 8], F32, kind="ExternalOutput")

    with tile.TileContext(nc) as tc:
        with (
            tc.tile_pool(name="persist", bufs=1) as persist,
            tc.tile_pool(name="scr", bufs=2) as scr,
            tc.tile_pool(name="stats", bufs=4) as stats,
            tc.tile_pool(name="psA", bufs=2, space=bass.MemorySpace.PSUM) as psA,
            tc.tile_pool(name="psP", bufs=1, space=bass.MemorySpace.PSUM) as psP,
            tc.tile_pool(name="psG", bufs=3, space=bass.MemorySpace.PSUM) as psG,
        ):
            bias_sb = persist.tile([1, 2 * D + 2 * DPJ], F8, tag="biasv")
            combo = persist.tile([P, _D1_COLS], F8, tag="combo")
            for k, (c0, c1) in enumerate(_D1_CHUNKS):
                nc.sync.dma_start(out=combo[:, c0:c1], in_=in_d[:, c0:c1])
                if k == 0:
                    nc.sync.dma_start(out=bias_sb[:], in_=bias_d[:])

            wpt = combo[:, _OFF_WPT:_OFF_WPT + _WPT_W].rearrange(
                "p (a o f) -> p a o f", a=KP, o=2)
            yts = [combo[:, _OFF_YT[r]:_OFF_YT[r] + _YT_W].rearrange(
                "p (a o m) -> p a o m", a=KP, o=2) for r in range(NB)]
            wtqs = [combo[:, _OFF_WTQ[q]:_OFF_WTQ[q] + _WTQ_W].rearrange(
                "p (a o f) -> p a o f", a=KP, o=2) for q in range(4)]
            # bias contraction pair: ones lhsT [1, 2, 128], rhs rows from
            # bias_sb ([b | 0] for x_pred, [b@P | 0] for the projection)
            ones_b = persist.tile([1, 2 * P], F8, tag="ones_b")
            nc.vector.memset(ones_b[:, :P], 1.0)
            nc.vector.memset(ones_b[:, P:], 0.0)
            ones_b3 = ones_b[:].rearrange("p (o m) -> p o m", o=2)
            bw3 = bias_sb[:, :2 * D].rearrange("p (o f) -> p o f", o=2)
            bp3 = bias_sb[:, 2 * D:].rearrange("p (o f) -> p o f", o=2)

            # p-state warmup: keep the PE busy while loads land so it is
            # at full clock when the real matmuls start
            warm = persist.tile([1, 2 * P], F8, tag="warm")
            nc.vector.memset(warm[:], 1.0)
            warm3 = warm[:].rearrange("p (o m) -> p o m", o=2)
            wps = psG.tile([P, P], F32, tag="g", name="warmps")
            for wi in range(70):
                nc.tensor.matmul(wps[:], warm3, warm3,
                                 start=(wi == 0), stop=(wi == 69),
                                 perf_mode=mybir.MatmulPerfMode.DoubleRow)

            xpn_lo = persist.tile([P, 4 * D], F8, tag="xpnlo")
            xpn_hi = persist.tile([P, 4 * D], F8, tag="xpnhi")

            def xpn3(rb):
                t = xpn_lo if rb < 4 else xpn_hi
                return t[:, (rb % 4) * D:(rb % 4 + 1) * D]

            def xpn_pair(a, cs, wd):
                # [128, 2, wd] DR operand from adjacent row blocks 2a, 2a+1
                t = xpn_lo if 2 * a < 4 else xpn_hi
                base = ((2 * a) % 4) * D
                return t[:, base:base + 2 * D].rearrange(
                    "p (r d) -> p r d", r=2)[:, :, cs:cs + wd]

            # per-evict-engine, per-half gram staging tiles (tile-granular
            # dependency tracking: a shared tile would WAW-chain engines, and
            # a streamed DMA read would WAR-block later writes)
            _echunks = [[ci for ci in range(len(_TRI_CHUNKS)) if ci % 2 == e]
                        for e in range(2)]
            _ehalves = []   # (engine, [chunk indices]) x4 in chunk order
            for e in range(2):
                cl = _echunks[e]
                _ehalves.append((e, cl[:3]))
                _ehalves.append((e, cl[3:]))
            gsbs = {}
            for hi, (e, cl) in enumerate(_ehalves):
                w = sum(_TRI_CHUNKS[ci][2] for ci in cl)
                gsbs[hi] = persist.tile([P, w], F8, tag=f"gsb{hi}",
                                        name=f"gsb{hi}")
            _chunk_home = {}
            for hi, (e, cl) in enumerate(_ehalves):
                off = 0
                for ci in cl:
                    _chunk_home[ci] = (hi, off)
                    off += _TRI_CHUNKS[ci][2]
            ones2 = persist.tile([P, 2], F8, tag="ones")
            nc.vector.memset(ones2[:], 1.0)
            ones3 = ones2[:].rearrange("p (o u) -> p o u", o=2)

            # Gram wave schedule: chunk index -> emitted after row block rb's
            # eviction chain (needs all rbs, so only emitted once xpn complete;
            # waves rotate through 3 PSUM tiles and 3 evict engines)
            kscale = float(1.0 / (WP_S * WP_S * XPN_S * XPN_S))

            def emit_gram_chunk(ci):
                pb, cs, wd = _TRI_CHUNKS[ci]
                g = psG.tile([P, MM_N], F32, tag="g")
                for a in range(KP):
                    nc.tensor.matmul(
                        g[:, :wd],
                        xpn_pair(a, pb * P, P),
                        xpn_pair(a, cs, wd),
                        start=(a == 0), stop=(a == KP - 1),
                        perf_mode=mybir.MatmulPerfMode.DoubleRow)
                hi, off = _chunk_home[ci]
                dst = gsbs[hi][:, off:off + wd]
                if ci % 2 == 0:
                    nc.vector.tensor_copy(dst, g[:, :wd])
                else:
                    nc.scalar.copy(dst, g[:, :wd])
                if ci == _ehalves[hi][1][-1]:
                    goff = sum(
                        sum(_TRI_CHUNKS[c][2] for c in _ehalves[h][1])
                        for h in range(hi))
                    nc.sync.dma_start(
                        out=gt_d[:, goff:goff + gsbs[hi].shape[1]],
                        in_=gsbs[hi][:])

            def mm_pair_chain(out_ap, yt, rhs4, brhs):
                for a in range(KP):
                    nc.tensor.matmul(
                        out_ap, yt[:, a, :, :], rhs4[a],
                        start=(a == 0), stop=False,
                        perf_mode=mybir.MatmulPerfMode.DoubleRow)
                nc.tensor.matmul(out_ap, ones_b3, brhs,
                                 start=False, stop=True,
                                 perf_mode=mybir.MatmulPerfMode.DoubleRow)

            rxps = {}
            for rb in range(NB):
                # projection matmuls for the pair first: starts both row-norm
                # chains early so evictions never stall the PSUM rotation
                if rb % 2 == 0:
                    pj = psP.tile([P, 2 * DPJ], F32, tag="proj")
                    for r2 in (rb, rb + 1):
                        mm_pair_chain(
                            pj[:, (r2 % 2) * DPJ:(r2 % 2 + 1) * DPJ],
                            yts[r2], [wpt[:, a, :, :] for a in range(KP)],
                            bp3)
                    for r2 in (rb, rb + 1):
                        pjs = pj[:, (r2 % 2) * DPJ:(r2 % 2 + 1) * DPJ]
                        pcp = scr.tile([P, DPJ], BF16, tag="pcp")
                        nc.vector.tensor_copy(pcp[:], pjs)
                        pscr = scr.tile([P, DPJ], BF16, tag="pscr")
                        ssp = stats.tile([P, 1], F32, tag="ssp")
                        nc.vector.scalar_tensor_tensor(
                            pscr[:], pcp[:], 1.0, pcp[:],
                            op0=mybir.AluOpType.mult,
                            op1=mybir.AluOpType.mult,
                            accum_out=ssp[:])
                        rsp = stats.tile([P, 1], F32, tag="rsp")
                        nc.vector.reciprocal(rsp[:], ssp[:])
                        rxp = stats.tile([P, 1], F32, tag="rxp")
                        nc.scalar.activation(
                            rxp[:], rsp[:],
                            mybir.ActivationFunctionType.Sqrt,
                            scale=float(1.0 / kscale))
                        rxps[r2] = rxp
                # x_pred in 4 column-quarter chains
                pp = psA.tile([P, D], F32, tag="pp")
                for cq in range(4):
                    mm_pair_chain(pp[:, cq * CQ:(cq + 1) * CQ], yts[rb],
                                  [wtqs[cq][:, a, :, :] for a in range(KP)],
                                  bw3[:, :, cq * CQ:(cq + 1) * CQ])
                nc.scalar.mul(xpn3(rb), pp[:], rxps[rb][:])
                if rb % 4 == 3:
                    nc.sync.dma_start(
                        out=xpn_d[:, (rb - 3) * D:(rb + 1) * D],
                        in_=(xpn_lo if rb < 4 else xpn_hi)[:])

            # T1 column sums first (uses one psG rotation slot briefly)
            t1p = psG.tile([P, MM_N], F32, tag="g")
            for a in range(KP):
                for pb in range(8):
                    nc.tensor.matmul(
                        t1p[:, pb:pb + 1],
                        xpn_pair(a, pb * P, P),
                        ones3,
                        start=(a == 0), stop=(a == KP - 1),
                        perf_mode=mybir.MatmulPerfMode.DoubleRow)
            t1sb = stats.tile([P, 8], F32, tag="t1sb")
            nc.vector.tensor_copy(t1sb[:], t1p[:, :8])
            nc.sync.dma_start(out=t1_d[:], in_=t1sb[:])

            for ci in range(len(_TRI_CHUNKS)):
                emit_gram_chunk(ci)

    nc.compile()
    return nc


def _build_dispatch2():
    nc = bacc.Bacc("TRN2", target_bir_lowering=False, debug=False,
                   num_devices=N_CORES)
    in_d = nc.dram_tensor("combo", [P, _D2_COLS], F8, kind="ExternalInput")
    # stat columns: [qA 0:8 | xsq 8:16 | pos 16:24 | m1 24:32 | qB 32:40],
    # split into two dram tensors so DVE-written (even rb) and Pool-written
    # (odd rb) columns live in different SBUF tiles (no cross-engine WAW)
    statd_d = nc.dram_tensor("statd", [P, 32], F32, kind="ExternalOutput")
    statqd_d = nc.dram_tensor("statqd", [P, 8], F32, kind="ExternalOutput")

    with tile.TileContext(nc) as tc:
        with (
            tc.tile_pool(name="persist", bufs=1) as persist,
            tc.tile_pool(name="scrd", bufs=4) as scrd,
            tc.tile_pool(name="scrp", bufs=4) as scrp,
            tc.tile_pool(name="psV", bufs=4, space=bass.MemorySpace.PSUM) as psV,
            tc.tile_pool(name="psB", bufs=4, space=bass.MemorySpace.PSUM) as psB,
        ):
            combo = persist.tile([P, _D2_COLS], F8, tag="combo")
            for (c0, c1) in _D2_CHUNKS:
                nc.sync.dma_start(out=combo[:, c0:c1], in_=in_d[:, c0:c1])

            eye = combo[:, _OFF_EYE:_OFF_EYE + P]
            t14 = combo[:, _OFF_T1:_OFF_T1 + KP * 2].rearrange(
                "p (a o u) -> p a o u", a=KP, o=2)
            xt4 = [combo[:, _OFF_PAIR + a * _PAIR_W:
                         _OFF_PAIR + a * _PAIR_W + _XT_W].rearrange(
                "p (o r) -> p o r", o=2) for a in range(KP)]
            rr4 = [combo[:, _OFF_PAIR + a * _PAIR_W + _XT_W:
                         _OFF_PAIR + (a + 1) * _PAIR_W].rearrange(
                "p (o f) -> p o f", o=2) for a in range(KP)]
            xpnt4 = [combo[:, _OFF_XPNT + a * _XT_W:
                           _OFF_XPNT + (a + 1) * _XT_W].rearrange(
                "p (o r) -> p o r", o=2) for a in range(KP)]

            statd = persist.tile([P, 32], F32, tag="statd")
            statqd = persist.tile([P, 8], F32, tag="statqd")

            def diag_extract(blk_ap, col, rb, late=False):
                s = scrd.tile([P, P], BF16, tag="dscr")
                dst = statqd if late else statd
                nc.vector.scalar_tensor_tensor(
                    s[:], blk_ap, 1.0, eye,
                    op0=mybir.AluOpType.mult, op1=mybir.AluOpType.mult,
                    accum_out=dst[:, col:col + 1])

            def block_group(rhs4, grp):
                """a-major [128,128] diag blocks for all 8 row blocks; all
                matmuls emitted before any extract (extracts read a whole
                tile, so an interleaved extract would WAR-serialize the
                remaining writes into that tile)."""
                bt0 = psB.tile([P, 4 * P], F32, tag="blk", name="bt0")
                bt1 = psB.tile([P, 4 * P], F32, tag="blk", name="bt1")
                tiles = [bt0, bt1]
                sls = [tiles[rb // 4][:, (rb % 4) * P:(rb % 4 + 1) * P]
                       for rb in range(NB)]
                for a in range(KP):
                    for rb in range(NB):
                        nc.tensor.matmul(
                            sls[rb], xt4[a][:, :, rb * P:(rb + 1) * P],
                            rhs4[a][:, :, rb * P:(rb + 1) * P],
                            start=(a == 0), stop=(a == KP - 1),
                            perf_mode=mybir.MatmulPerfMode.DoubleRow)
                for rb in range(NB):
                    diag_extract(sls[rb], grp * 8 + rb, rb)

            # ---- xsq = diag(X X^T) and m1 = X.T1, gated only on xT pairs ----
            block_group(xt4, 1)
            m1t = psB.tile([P, 4 * P], F32, tag="blk", name="m1t")
            for a in range(KP):
                for rb in range(NB):
                    nc.tensor.matmul(
                        m1t[:, rb:rb + 1],
                        xt4[a][:, :, rb * P:(rb + 1) * P], t14[:, a],
                        start=(a == 0), stop=(a == KP - 1),
                        perf_mode=mybir.MatmulPerfMode.DoubleRow)
            nc.vector.tensor_copy(statd[:, 24:32], m1t[:, :8])

            # ---- v^T = R @ x^T per pblock (ACT evicts to fp8), with the
            # q = diag(X V^T) half-chains and pos blocks interleaved so only
            # the last q half-chain gates on the final eviction ----
            vts = [persist.tile([P, 2 * NS], F8, tag=f"vt{a}",
                                name=f"vt{a}") for a in range(KP)]
            vt4 = [vts[a][:].rearrange("p (o r) -> p o r", o=2)
                   for a in range(KP)]

            def mm1_pb(pb):
                # two half-width accumulation chains per pblock, each evicted
                # as soon as it completes; a vt pair-tile is written by one
                # engine only (ACT for pairs 0/2, DVE for pairs 1/3) to avoid
                # cross-engine WAW chaining on the tile
                for c in range(NS // MM_N):
                    pv = psV.tile([P, MM_N], F32, tag="vt")
                    for a in range(KP):
                        nc.tensor.matmul(
                            pv[:], rr4[a][:, :, pb * P:(pb + 1) * P],
                            xt4[a][:, :, c * MM_N:(c + 1) * MM_N],
                            start=(a == 0), stop=(a == KP - 1),
                            perf_mode=mybir.MatmulPerfMode.DoubleRow)
                    dst = vt4[pb // 2][:, pb % 2, c * MM_N:(c + 1) * MM_N]
                    if pb < 2:
                        nc.vector.tensor_copy(dst, pv[:])
                    else:
                        nc.scalar.copy(dst, pv[:])

            def q_step(sls, a, a0):
                for rb in range(NB):
                    nc.tensor.matmul(
                        sls[rb], xt4[a][:, :, rb * P:(rb + 1) * P],
                        vt4[a][:, :, rb * P:(rb + 1) * P],
                        start=(a == a0), stop=(a == a0 + 1),
                        perf_mode=mybir.MatmulPerfMode.DoubleRow)

            def q_tiles(nm):
                qta = psB.tile([P, 4 * P], F32, tag="blk", name="qta")
                qtb = psB.tile([P, 4 * P], F32, tag="blk", name="qtb")
                return [[qta, qtb][rb // 4][:, (rb % 4) * P:(rb % 4 + 1) * P]
                        for rb in range(NB)]

            mm1_pb(0); mm1_pb(1); mm1_pb(2); mm1_pb(3); mm1_pb(4)
            slsA = q_tiles("qA")
            q_step(slsA, 0, 0); q_step(slsA, 1, 0)   # pairs a0,a1 (pbs 0-3)
            for rb in range(NB):
                diag_extract(slsA[rb], 0 * 4 + rb // 2, rb)
            block_group(xpnt4, 2)                    # pos (xpnT landed)
            # early stat flush overlaps the mm1/q tail
            nc.sync.dma_start(out=statd_d[:], in_=statd[:])
            mm1_pb(5)
            slsB = q_tiles("qB")
            q_step(slsB, 2, 2)                       # pair a2 (pbs 4,5)
            mm1_pb(6); mm1_pb(7)
            q_step(slsB, 3, 2)                       # pair a3 (pbs 6,7)
            for rb in range(NB):
                diag_extract(slsB[rb], rb, rb, late=True)

            nc.sync.dma_start(out=statqd_d[:], in_=statqd[:])

    nc.compile()
    return nc


_NC1 = None
_NC2 = None
_NCFB = None


def _programs():
    global _NC1, _NC2
    if _NC1 is None:
        _NC1 = _build_dispatch1()
    if _NC2 is None:
        _NC2 = _build_dispatch2()
    return _NC1, _NC2


def _pair_swizzle_T(at, f):
    """[K, f] (K = 256*npair) -> pair-major [128, npair*2*f] fp8."""
    k = at.shape[0]
    npair = k // 256
    return np.ascontiguousarray(
        at.reshape(npair, 2, P, f).transpose(2, 0, 1, 3).reshape(P, npair * 2 * f))


def _projection():
    rng = np.random.default_rng(12345)
    # rademacher +-1/sqrt(DPJ) preserves row norms in expectation
    return (rng.integers(0, 2, size=(D, DPJ)).astype(np.float32) * 2.0
            - 1.0) / np.float32(np.sqrt(DPJ))


def kernel(x, y, W, b, _timing=None):
    assert x.shape == (N, D) and y.shape == (N, D)
    assert W.shape == (D, D) and b.shape == (D,)
    nc1, nc2 = _programs()
    core_ids = list(range(N_CORES))

    x = np.asarray(x, dtype=np.float32)
    y = np.asarray(y, dtype=np.float32)
    W = np.asarray(W, dtype=np.float32)
    b = np.asarray(b, dtype=np.float32)

    # ---- dispatch 1 inputs ----
    A8 = np.ascontiguousarray(W.T).astype(NP_F8)      # [D, D]
    Pm = _projection()
    Ap8 = ((W.T @ Pm) * np.float32(WP_S)).astype(NP_F8)
    wpT_sw = _pair_swizzle_T(Ap8, DPJ)
    bias_in = np.zeros((1, 2 * D + 2 * DPJ), dtype=NP_F8)
    bias_in[0, :D] = b.astype(NP_F8)
    bias_in[0, 2 * D:2 * D + DPJ] = ((b @ Pm) * np.float32(WP_S)).astype(NP_F8)

    y8 = y.astype(NP_F8)
    combo1_shared = np.zeros((P, _D1_COLS), dtype=NP_F8)
    combo1_shared[:, _OFF_WPT:_OFF_WPT + _WPT_W] = wpT_sw
    for q in range(4):
        wq = _pair_swizzle_T(np.ascontiguousarray(A8[:, q * CQ:(q + 1) * CQ]),
                             CQ)
        combo1_shared[:, _OFF_WTQ[q]:_OFF_WTQ[q] + _WTQ_W] = wq

    in_maps1 = []
    for i in range(N_CORES):
        sl = slice(i * NS, (i + 1) * NS)
        yT = np.ascontiguousarray(y8[sl].T)           # [D, NS]
        cm = combo1_shared.copy()
        for r in range(NB):
            cm[:, _OFF_YT[r]:_OFF_YT[r] + _YT_W] = _pair_swizzle_T(
                np.ascontiguousarray(yT[:, r * P:(r + 1) * P]), P)
        in_maps1.append({"combo": cm, "biasv": bias_in})
    r1 = run_bass_kernel_spmd(nc1, in_maps1, core_ids)
    if _timing is not None:
        _timing["d1"] = r1.exec_time_ns

    # ---- host glue: assemble M2, R, T1; build transposed operands ----
    xpn8 = np.concatenate(
        [r1.results[i]["xpn"].reshape(P, NB, D).transpose(1, 0, 2)
         .reshape(NS, D) for i in range(N_CORES)], axis=0)  # [N, D], 16*xpn
    G = np.zeros((D, D), dtype=np.float32)
    # gtri layout: per-engine halves [e0h0 | e0h1 | e1h0 | e1h1]
    _ech = [[ci for ci in range(len(_TRI_CHUNKS)) if ci % 2 == e]
            for e in range(2)]
    tri_order = []
    for e in range(2):
        tri_order += _ech[e][:3] + _ech[e][3:]
    tri_off = {}
    _o = 0
    for ci in tri_order:
        tri_off[ci] = _o
        _o += _TRI_CHUNKS[ci][2]
    for i in range(N_CORES):
        gt = r1.results[i]["gtri"].astype(np.float32)
        for ci, (pb, cs, wd) in enumerate(_TRI_CHUNKS):
            G[pb * P:(pb + 1) * P, cs:cs + wd] += gt[:, tri_off[ci]:
                                                     tri_off[ci] + wd]
    for pb in range(8):  # mirror lower triangle
        for qb in range(pb):
            G[pb * P:(pb + 1) * P, qb * P:(qb + 1) * P] = \
                G[qb * P:(qb + 1) * P, pb * P:(pb + 1) * P].T
    M2 = G / np.float32(XPN_S * XPN_S)
    c0 = float(np.trace(M2)) / D
    R16 = (M2 - c0 * np.eye(D, dtype=np.float32)) * np.float32(R_S)
    rr_sw = _pair_swizzle_T(R16.astype(NP_F8), D)
    t1v = np.zeros((D,), dtype=np.float32)
    for i in range(N_CORES):
        t1v += r1.results[i]["t1v"].T.reshape(D)   # 16*T1
    t1_sw = _pair_swizzle_T(t1v.astype(NP_F8).reshape(KP * 256, 1), 1)\
        .reshape(P, KP * 2)

    x8 = x.astype(NP_F8)
    eye8 = np.eye(P, dtype=NP_F8)
    in_maps2 = []
    for i in range(N_CORES):
        sl = slice(i * NS, (i + 1) * NS)
        xT_sw = _pair_swizzle_T(np.ascontiguousarray(x8[sl].T), NS)
        xpnT_sw = _pair_swizzle_T(np.ascontiguousarray(xpn8[sl].T), NS)
        cm = np.zeros((P, _D2_COLS), dtype=NP_F8)
        cm[:, _OFF_EYE:_OFF_EYE + P] = eye8
        cm[:, _OFF_T1:_OFF_T1 + KP * 2] = t1_sw
        for a in range(KP):
            cm[:, _OFF_PAIR + a * _PAIR_W:
               _OFF_PAIR + a * _PAIR_W + _XT_W] = \
                xT_sw[:, a * _XT_W:(a + 1) * _XT_W]
            cm[:, _OFF_PAIR + a * _PAIR_W + _XT_W:
               _OFF_PAIR + (a + 1) * _PAIR_W] = \
                rr_sw[:, a * _RR_W:(a + 1) * _RR_W]
        cm[:, _OFF_XPNT:_D2_COLS] = xpnT_sw
        in_maps2.append({"combo": cm})
    r2 = run_bass_kernel_spmd(nc2, in_maps2, core_ids)
    if _timing is not None:
        _timing["d2"] = r2.exec_time_ns

    # ---- host final assembly ----
    qv, xsq, posr, m1r = [], [], [], []
    for i in range(N_CORES):
        std = r2.results[i]["statd"].astype(np.float64)
        sqd = r2.results[i]["statqd"].astype(np.float64)
        qv.append(std[:, 0:8].T.reshape(NS) + sqd.T.reshape(NS))
        xsq.append(std[:, 8:16].T.reshape(NS))
        posr.append(std[:, 16:24].T.reshape(NS))
        m1r.append(std[:, 24:32].T.reshape(NS))
    qv = np.concatenate(qv)      # 16 * x R x (R-residual quadratic form)
    xsq = np.concatenate(xsq)    # ||x||^2
    posr = np.concatenate(posr)  # 16 * x . xpn
    m1r = np.concatenate(m1r)    # 16 * x . T1

    rx2 = 1.0 / xsq
    rx = np.sqrt(rx2)
    m2 = c0 + qv / R_S * rx2
    m1 = m1r / XPN_S * rx
    se = N + m1 + m2 / 2
    neg = np.log(se)
    pos = posr / XPN_S * rx
    loss = np.mean(neg) - np.mean(pos)

    # a-posteriori certificate for the 2nd-order truncation
    smax = np.sqrt(np.maximum(m2, 0.0))
    resid = np.maximum(m2, 0.0) ** 1.5 / 6.0 * np.exp(smax)
    worst = np.max(resid / np.maximum(se - resid, 1.0))
    if not np.isfinite(loss) or worst > 8e-3 * abs(loss):
        neg = _exact_neg_fallback(x8, xpn8, rx, _timing)
        loss = np.mean(neg) - np.mean(pos)

    return np.asarray(loss, dtype=np.float32)


# ---------------------------------------------------------------------------
# exact exp/logsumexp fallback (never triggered for the reference input
# distribution; kept for certified correctness on adversarial inputs)
# ---------------------------------------------------------------------------

def _build_fallback():
    JC_W = 2048
    N_JC = N // JC_W
    NTP = KP
    nc = bacc.Bacc("TRN2", target_bir_lowering=False, debug=False,
                   num_devices=N_CORES)
    xT_d = nc.dram_tensor("xT", [P, D // P * NS], F8, kind="ExternalInput")
    xpnT_d = nc.dram_tensor("xpnT", [P, D // P * N], F8, kind="ExternalInput")
    rx_d = nc.dram_tensor("rxv", [P, NB], F32, kind="ExternalInput")
    neg_d = nc.dram_tensor("negv", [P, NB], F32, kind="ExternalOutput")
    DT = D // P
    with tile.TileContext(nc) as tc:
        with (
            tc.tile_pool(name="persist", bufs=1) as persist,
            tc.tile_pool(name="esc", bufs=2) as escp,
            tc.tile_pool(name="psum", bufs=2, space=bass.MemorySpace.PSUM) as psum,
        ):
            rx_sb = persist.tile([P, NB], F32, tag="rx")
            nc.gpsimd.dma_start(out=rx_sb[:], in_=rx_d[:])
            xib = []
            for ib in range(NB):
                xt = persist.tile([P, DT * P], F8, tag=f"xib{ib}")
                nc.gpsimd.dma_start(
                    out=xt[:], in_=xT_d[:, ib * DT * P:(ib + 1) * DT * P])
                xib.append(xt)
            separts = persist.tile([P, NB * N_JC], F32, tag="separts")
            for jc in range(N_JC):
                xp_tp = []
                for tp in range(NTP):
                    base = (jc * NTP + tp) * 2 * JC_W
                    xp = persist.tile([P, 2 * JC_W], F8, tag=f"xpnT{jc}_{tp}")
                    nc.sync.dma_start(out=xp[:],
                                      in_=xpnT_d[:, base:base + 2 * JC_W])
                    xp_tp.append(xp)
                for ib in range(NB):
                    x3 = xib[ib][:].rearrange("p (t m) -> p t m", t=DT)
                    ps = psum.tile([P, JC_W], F32, tag="ps")
                    for tp in range(NTP):
                        lhs3 = x3[:, 2 * tp:2 * tp + 2, :]
                        rhs3 = xp_tp[tp][:].rearrange("p (o c) -> p o c", o=2)
                        for c in range(JC_W // MM_N):
                            nc.tensor.matmul(
                                ps[:, c * MM_N:(c + 1) * MM_N],
                                lhs3,
                                rhs3[:, :, c * MM_N:(c + 1) * MM_N],
                                start=(tp == 0), stop=(tp == NTP - 1),
                                perf_mode=mybir.MatmulPerfMode.DoubleRow)
                    esc = escp.tile([P, JC_W], BF16, tag="esc")
                    nc.scalar.activation(
                        esc[:], ps[:], mybir.ActivationFunctionType.Exp,
                        scale=rx_sb[:, ib:ib + 1],
                        accum_out=separts[:, ib * N_JC + jc:
                                          ib * N_JC + jc + 1])
            se_all = persist.tile([P, NB], F32, tag="se_all")
            nc.vector.reduce_sum(
                se_all[:], separts[:].rearrange("p (i j) -> p i j", j=N_JC),
                axis=mybir.AxisListType.X)
            neg_sb = persist.tile([P, NB], F32, tag="neg_sb")
            nc.scalar.activation(neg_sb[:], se_all[:],
                                 mybir.ActivationFunctionType.Ln)
            nc.sync.dma_start(out=neg_d[:], in_=neg_sb[:])
    nc.compile()
    return nc


def _exact_neg_fallback(x8, xpn8, rx, _timing):
    global _NCFB
    if _NCFB is None:
        _NCFB = _build_fallback()
    DT = D // P
    # xpn8 is 16*xpn; fold 1/16 into the exp scale
    xpnT = np.ascontiguousarray(xpn8.T)  # [D, N]
    xpnT_sw = np.ascontiguousarray(
        xpnT.reshape(KP, 2, P, N // 2048, 2048).transpose(2, 3, 0, 1, 4)
        .reshape(P, DT * N))
    in_maps = []
    for i in range(N_CORES):
        sl = slice(i * NS, (i + 1) * NS)
        rx_sw = np.ascontiguousarray(
            (rx[sl] / XPN_S).astype(np.float32).reshape(NB, P).T)
        xT8 = np.ascontiguousarray(x8[sl].T)
        xT_sw = np.ascontiguousarray(
            xT8.reshape(DT, P, NB, P).transpose(1, 2, 0, 3)
            .reshape(P, DT * NS))
        in_maps.append({"xT": xT_sw, "xpnT": xpnT_sw, "rxv": rx_sw})
    r = run_bass_kernel_spmd(_NCFB, in_maps, list(range(N_CORES)))
    if _timing is not None:
        _timing["dfb"] = r.exec_time_ns
    return np.concatenate(
        [r.results[i]["negv"].T.reshape(NS) for i in range(N_CORES)])
